# revision 29
# baseline (speedup 1.0000x reference)
"""GQA attention (S=2048, D=4096, 32 Q heads / 8 KV heads, RoPE, full attn)
distributed over 8 Trainium2 NeuronCores.

Strategy (tensor-parallel by heads; AllGather of normalized attention before
the output projection):
  - core c owns Q heads 4c..4c+3 and KV head c (GQA groups align with cores).
  - all GEMMs bf16 (stationary and moving), f32 PSUM accumulation.
  - projections as transposed GEMMs QT/KT/VT [chan, tok]: sweeps PAIRED so
    consecutive matmuls alternate between two PSUM banks (single-bank
    back-to-back accumulation loses ~70ns/MM of drain overlap); chunks 0-2
    use a dedicated 4-bank ring, chunk 3's K/V + Q sweeps share a 2-bank
    ring with the attention-chunk-0 fillers.
  - RoPE via DVE muls (u=p*cs1, v=p*cs2) + deferred PE mix-matmuls flushed
    mid-next-sweep (never blocks the PE on the DVE); V transposed by
    SBUF->SBUF transposing DMAs.
  - attention per 512-token q-chunk, per head: scores ST=[k,q] per k-tile
    (3-bank ring), exp on ScalarE -> bf16, z on DVE (running f32 sum +
    ones-matmul partition reduce), PV accumulated over 16 k-tiles;
    normalize (1/z via ScalarE ln/exp, ones-broadcast matmul, DVE mul)
    deferred into the NEXT head's kt stream so the PE never waits.
  - the attention inner loop is exp-paced, so independent filler matmuls
    (wo-GEMM quarters of earlier chunks / last chunk's Q sweeps) fill the
    in-order PE queue between score/PV pairs.
  - AllGather per (chunk, head-pair) for chunks 0-2 ([256,512]bf16 ->
    [2048,512]) and per HEAD for chunk 3 ([128,512] -> [1024,512]) so the
    tail wo GEMM never waits; gathered tiles are DMA'd in half-waves
    pre-issued as soon as each collective is launched.
Host side only reshapes/transposes/casts inputs and concatenates outputs.
"""
import sys

import numpy as np
import ml_dtypes

_BF16 = ml_dtypes.bfloat16

for _p in ("/root/.axon_site/_ro/trn_rl_repo", "/opt/trn_rl_repo"):
    if _p not in sys.path:
        sys.path.append(_p)

import concourse.bass as bass
import concourse.tile as tile
from concourse import mybir
from concourse.bass_utils import run_bass_kernel_spmd

N_CORES = 8
S = 2048
D = 4096
HD = 128
N_QH = 4          # Q heads per core
N_KT = S // 128   # 16 k-tiles
N_TC = S // 512   # 4 token chunks
N_KC = D // 128   # 32 contraction tiles
F32 = mybir.dt.float32
BF = mybir.dt.bfloat16

_NC_CACHE = {}


def _split_multi_waits(nc):
    """This container's walrus accepts only ONE sync-wait per instruction
    encoding; hoist extra waits onto fresh single-wait NoOps placed before
    the instruction on the same engine."""
    n = 0
    for fn in nc.m.functions:
        for bb in fn.blocks:
            new_insts = []
            changed = False
            for ins in bb.instructions:
                si = ins.sync_info
                waits = list(si.on_wait) if si is not None else []
                if len(waits) > 1:
                    for w in waits[:-1]:
                        n += 1
                        nop = mybir.InstNoOp(name=f"WSPL-{n}", ins=[], outs=[])
                        nop.engine = ins.engine
                        nop.sync_info = mybir.SyncInfo(on_wait=[w], on_update=[])
                        new_insts.append(nop)
                    si.on_wait = waits[-1:]
                    changed = True
                new_insts.append(ins)
            if changed:
                bb.instructions = new_insts
    return n


def _build():
    nc = bass.Bass()

    xt = nc.dram_tensor("xt", [N_KC, 128, S], BF, kind="ExternalInput")
    wqt = nc.dram_tensor("wqt", [128, N_KC, 512], BF, kind="ExternalInput")
    wkt = nc.dram_tensor("wkt", [128, N_KC, HD], BF, kind="ExternalInput")
    wvt = nc.dram_tensor("wvt", [128, N_KC, HD], BF, kind="ExternalInput")
    wot = nc.dram_tensor("wot", [128, N_KC, 512], BF, kind="ExternalInput")
    cs1 = nc.dram_tensor("cs1", [HD, S], BF, kind="ExternalInput")
    cs2 = nc.dram_tensor("cs2", [HD, S], BF, kind="ExternalInput")
    mix1 = nc.dram_tensor("mix1", [HD, HD], BF, kind="ExternalInput")
    mix2 = nc.dram_tensor("mix2", [HD, HD], BF, kind="ExternalInput")
    onesc = nc.dram_tensor("onesc", [HD, 1], F32, kind="ExternalInput")
    onesr = nc.dram_tensor("onesr", [1, HD], BF, kind="ExternalInput")
    out_ext = nc.dram_tensor("out", [S, 512], F32, kind="ExternalOutput")

    # chunks 0-2: AllGather per head-pair; chunk 3: per head
    ag_in = {}
    ag_out = {}
    for qc in range(3):
        for hp in range(2):
            ag_in[(qc, hp)] = nc.dram_tensor(f"agi{qc}_{hp}", [256, 512], BF)
            ag_out[(qc, hp)] = nc.dram_tensor(
                f"ago{qc}_{hp}", [2048, 512], BF, addr_space="Shared"
            )
    for h in range(N_QH):
        ag_in[(3, h)] = nc.dram_tensor(f"agi3_{h}", [128, 512], BF)
        ag_out[(3, h)] = nc.dram_tensor(
            f"ago3_{h}", [1024, 512], BF, addr_space="Shared"
        )

    with tile.TileContext(nc) as tc:
        with (
            tc.tile_pool(name="const", bufs=1) as constp,
            tc.tile_pool(name="persist", bufs=1) as persist,
            tc.tile_pool(name="xtp", bufs=12) as xtp,
            tc.tile_pool(name="uv", bufs=2) as uvp,
            tc.tile_pool(name="vt", bufs=2) as vtp,
            tc.tile_pool(name="ep", bufs=4) as ep,
            tc.tile_pool(name="zp", bufs=2) as zpool,
            tc.tile_pool(name="small", bufs=4) as smallp,
            tc.tile_pool(name="at", bufs=3) as atp,
            tc.tile_pool(name="rhs", bufs=5) as rhsp,
            tc.tile_pool(name="fout", bufs=2) as foutp,
        ):
            # ---- constants ----
            cs1_sb = constp.tile([HD, S], BF)
            cs2_sb = constp.tile([HD, S], BF)
            mix1_sb = constp.tile([HD, HD], BF)
            mix2_sb = constp.tile([HD, HD], BF)
            onesc_sb = constp.tile([HD, 1], F32)
            onesr_sb = constp.tile([1, HD], BF)
            nc.gpsimd.dma_start(out=cs1_sb[:], in_=cs1[:])
            nc.gpsimd.dma_start(out=cs2_sb[:], in_=cs2[:])
            nc.gpsimd.dma_start(out=mix1_sb[:], in_=mix1[:])
            nc.gpsimd.dma_start(out=mix2_sb[:], in_=mix2[:])
            nc.gpsimd.dma_start(out=onesc_sb[:], in_=onesc[:])
            nc.gpsimd.dma_start(out=onesr_sb[:], in_=onesr[:])

            # ---- weights (wq and wo phases don't overlap: scoped pools) ----
            wk_sb = persist.tile([128, N_KC, HD], BF)
            wv_sb = persist.tile([128, N_KC, HD], BF)
            nc.sync.dma_start(out=wk_sb[:], in_=wkt[:])
            nc.sync.dma_start(out=wv_sb[:], in_=wvt[:])

            # ---- persistent activations ----
            qt_sb = persist.tile([128, N_QH, S], BF)
            kt_sb = persist.tile([128, S], BF)
            v_sb = persist.tile([128, N_KT, HD], BF)

            xt_tiles = {}   # (c, g) -> sbuf tile [128, 4, 512]

            def load_xt_group(c, g):
                t = xtp.tile([128, 4, 512], BF, name=f"xt{c}_{g}", tag="xt")
                nc.sync.dma_start(
                    out=t[:],
                    in_=xt[g * 4:(g + 1) * 4, :, c * 512:(c + 1) * 512].rearrange(
                        "g p n -> p g n"
                    ),
                )
                xt_tiles[(c, g)] = t

            for g in range(8):
                load_xt_group(0, g)

            # rope stage 2 (PE mix matmuls + copy) is deferred and flushed
            # mid-next-sweep, so the PE never waits on the DVE muls.
            pending_rope = []

            def flush_rope(pool):
                while pending_rope:
                    u, v, dst, key = pending_rope.pop(0)
                    rps = pool.tile([128, 512], F32, name=f"rps{key}", tag=pool._ropetag)
                    nc.tensor.matmul(rps[:], mix1_sb[:], u[:], start=True, stop=False)
                    nc.tensor.matmul(rps[:], mix2_sb[:], v[:], start=False, stop=True)
                    nc.scalar.copy(dst, rps[:])

            def rope_stage1(acc, dst, t0, key):
                u = uvp.tile([128, 512], BF, name=f"u{key}", tag="u")
                v = uvp.tile([128, 512], BF, name=f"v{key}", tag="v")
                nc.vector.tensor_mul(u[:], acc[:], cs1_sb[:, t0:t0 + 512])
                nc.vector.tensor_mul(v[:], acc[:], cs2_sb[:, t0:t0 + 512])
                pending_rope.append((u, v, dst, key))

            def proj_drain(acc, c, kind, h):
                t0 = c * 512
                if kind == "q":
                    rope_stage1(acc, qt_sb[:, h, t0:t0 + 512], t0, f"q{c}_{h}")
                elif kind == "k":
                    rope_stage1(acc, kt_sb[:, t0:t0 + 512], t0, f"k{c}")
                else:
                    vt_t = vtp.tile([128, 512], BF, name=f"vt{c}", tag="vt")
                    nc.scalar.copy(vt_t[:], acc[:])
                    for g in range(4):
                        nc.sync.dma_start_transpose(
                            out=v_sb[:, c * 4 + g, :],
                            in_=vt_t[:, g * 128:(g + 1) * 128],
                        )

            def wslice_fn(kind, h):
                if kind == "q":
                    return lambda kc: wq_sb[:, kc, h * 128:(h + 1) * 128]
                if kind == "k":
                    return lambda kc: wk_sb[:, kc, :]
                return lambda kc: wv_sb[:, kc, :]

            def proj_sweep(pp, rope_pool, c, kind, h=0):
                """One 32-MM projection sweep into one PSUM bank."""
                acc = pp.tile([128, 512], F32, name=f"acc_{kind}{c}_{h}", tag="pacc")
                ws = wslice_fn(kind, h)
                for kc in range(N_KC):
                    if kc == 8:
                        flush_rope(rope_pool)
                    nc.tensor.matmul(
                        acc[:], ws(kc), xt_tiles[(c, kc // 4)][:, kc % 4, :],
                        start=(kc == 0), stop=(kc == N_KC - 1),
                    )
                proj_drain(acc, c, kind, h)

            def proj_sweep_gen(pp, rope_pool, c, kind, h=0):
                """proj_sweep as a filler generator: 16 yields of 2 MMs."""
                acc = pp.tile([128, 512], F32, name=f"acc_{kind}{c}_{h}", tag="pacc")
                ws = wslice_fn(kind, h)
                for kp in range(16):
                    if kp == 4:
                        flush_rope(rope_pool)
                    for kc in (2 * kp, 2 * kp + 1):
                        nc.tensor.matmul(
                            acc[:], ws(kc), xt_tiles[(c, kc // 4)][:, kc % 4, :],
                            start=(kc == 0), stop=(kc == N_KC - 1),
                        )
                    if kp < 15:
                        yield
                proj_drain(acc, c, kind, h)
                yield

            # ---------- wo GEMM machinery ----------
            wo_state = {}
            cur_pools = {}   # phase-scoped PSUM pools for attention

            def load_rhs_halves(qc, hp):
                """DMA one gathered wave into two [128,8,512] half tiles."""
                halves = []
                for half in range(2):
                    r = rhsp.tile(
                        [128, 8, 512], BF, name=f"rhs{qc}_{hp}_{half}", tag="rhs"
                    )
                    nc.sync.dma_start(
                        out=r[:],
                        in_=ag_out[(qc, hp)][half * 1024:(half + 1) * 1024, :]
                        .rearrange("(t p) n -> p t n", p=128),
                    )
                    halves.append(r)
                wo_state[(qc, hp)] = halves

            def load_rhs3(h):
                r = rhsp.tile([128, 8, 512], BF, name=f"rhs3_{h}", tag="rhs")
                nc.sync.dma_start(
                    out=r[:],
                    in_=ag_out[(3, h)].rearrange("(t p) n -> p t n", p=128),
                )
                wo_state[(3, h)] = r

            def wo_quarter_gen(qc, wop, quarter):
                """wo GEMM for chunk qc (0..2), one quarter: 16 yields x 2 MMs.
                quarter 0: fps01 += wave0      quarter 1: fps01 += wave1, fout
                quarter 2: fps23 += wave0      quarter 3: fps23 += wave1, fout"""
                q0 = qc * 512
                hp = quarter % 2
                qsp = quarter // 2
                halves = wo_state[(qc, hp)]
                if hp == 0:
                    fps = [
                        wop.tile(
                            [128, 512], F32, name=f"f{qc}_{qsp}_{j}", tag=f"f{j}"
                        )
                        for j in range(2)
                    ]
                    wo_state[(qc, "fps", qsp)] = fps
                fps = wo_state[(qc, "fps", qsp)]
                for i in range(16):
                    ci, jj = i // 2, i % 2
                    hk = 4 * ci + 2 * hp + jj
                    rhs = halves[0] if ci < 4 else halves[1]
                    li = (ci % 4) * 2 + jj
                    for j in range(2):
                        qs = qsp * 2 + j
                        nc.tensor.matmul(
                            fps[j][:],
                            rhs[:, li, qs * 128:(qs + 1) * 128],
                            wo_sb[:, hk, :],
                            start=(hp == 0 and i == 0),
                            stop=(hp == 1 and i == 15),
                        )
                    if i < 15:
                        yield
                if hp == 1:
                    for j in range(2):
                        qs = qsp * 2 + j
                        f_sb = foutp.tile(
                            [128, 512], F32, name=f"fs{qc}_{qsp}_{j}", tag="fs"
                        )
                        nc.scalar.copy(f_sb[:], fps[j][:])
                        nc.sync.dma_start(
                            out=out_ext[q0 + qs * 128:q0 + (qs + 1) * 128, :],
                            in_=f_sb[:],
                        )
                yield

            def wo3_pass(wop, qsp):
                """Last chunk: one fps pair accumulated across 4 head-waves."""
                q0 = 3 * 512
                fps = [
                    wop.tile([128, 512], F32, name=f"f3_{qsp}_{j}", tag=f"f{j}")
                    for j in range(2)
                ]
                for h in range(N_QH):
                    rhs = wo_state[(3, h)]
                    for ci in range(8):
                        hk = 4 * ci + h
                        for j in range(2):
                            qs = qsp * 2 + j
                            nc.tensor.matmul(
                                fps[j][:],
                                rhs[:, ci, qs * 128:(qs + 1) * 128],
                                wo_sb[:, hk, :],
                                start=(h == 0 and ci == 0),
                                stop=(h == N_QH - 1 and ci == 7),
                            )
                for j in range(2):
                    qs = qsp * 2 + j
                    f_sb = foutp.tile(
                        [128, 512], F32, name=f"fs3_{qsp}_{j}", tag="fs"
                    )
                    nc.scalar.copy(f_sb[:], fps[j][:])
                    nc.sync.dma_start(
                        out=out_ext[q0 + qs * 128:q0 + (qs + 1) * 128, :],
                        in_=f_sb[:],
                    )

            def run_filler(f, n=1):
                if f is None:
                    return
                for _ in range(n):
                    try:
                        next(f)
                    except StopIteration:
                        break

            def ag_launch(qc, part):
                nc.gpsimd.collective_compute(
                    "AllGather",
                    mybir.AluOpType.bypass,
                    replica_groups=[list(range(N_CORES))],
                    ins=[ag_in[(qc, part)][:].opt()],
                    outs=[ag_out[(qc, part)][:].opt()],
                )

            class NormTail:
                """Deferred per-head softmax normalization, emitted inside the
                NEXT head's kt stream (kt2: z-reduce + 1/z; kt8: broadcast,
                normalize, store, collective launch + gathered-wave DMA)."""

                def __init__(self, qc, h, pv, z_acc):
                    self.qc, self.h, self.pv, self.z_acc = qc, h, pv, z_acc

                def stage_a(self):
                    qc, h = self.qc, self.h
                    zr = cur_pools["pzb"].tile([1, 512], F32, name=f"zr{qc}_{h}", tag="zb")
                    nc.tensor.matmul(
                        zr[:], onesc_sb[:], self.z_acc[:], start=True, stop=True
                    )
                    lnz = smallp.tile([1, 512], F32, name=f"ln{qc}_{h}", tag="lnz")
                    nc.scalar.activation(
                        out=lnz[:], in_=zr[:],
                        func=mybir.ActivationFunctionType.Ln,
                    )
                    self.invz = smallp.tile([1, 512], BF, name=f"iz{qc}_{h}", tag="iz")
                    nc.scalar.activation(
                        out=self.invz[:], in_=lnz[:],
                        func=mybir.ActivationFunctionType.Exp, scale=-1.0,
                    )

                def stage_b(self):
                    qc, h = self.qc, self.h
                    bc = cur_pools["pzb"].tile([128, 512], F32, name=f"bc{qc}_{h}", tag="zb")
                    nc.tensor.matmul(
                        bc[:], onesr_sb[:], self.invz[:], start=True, stop=True
                    )
                    bc_sb = smallp.tile([128, 512], BF, name=f"bcs{qc}_{h}", tag="bcs")
                    nc.scalar.copy(bc_sb[:], bc[:])
                    at_sb = atp.tile([128, 512], BF, name=f"at{qc}_{h}", tag="at")
                    nc.vector.tensor_mul(at_sb[:], self.pv[:], bc_sb[:])
                    if qc < 3:
                        hp, j = h // 2, h % 2
                        nc.gpsimd.dma_start(
                            out=ag_in[(qc, hp)][j * 128:(j + 1) * 128, :],
                            in_=at_sb[:],
                        )
                        if j == 1:
                            ag_launch(qc, hp)
                            if hp == 1:
                                # wave1 halves: ring slot is free by now
                                load_rhs_halves(qc, 1)
                    else:
                        nc.gpsimd.dma_start(out=ag_in[(3, h)][:], in_=at_sb[:])
                        ag_launch(3, h)
                        load_rhs3(h)

            def emit_ev(qc, h, kt, stps, pv, z_acc):
                e_t = ep.tile([128, 512], BF, name=f"e{qc}_{h}_{kt}", tag="e")
                nc.scalar.activation(
                    out=e_t[:], in_=stps[kt][:],
                    func=mybir.ActivationFunctionType.Exp,
                )
                nc.tensor.matmul(
                    pv[:], v_sb[:, kt, :], e_t[:],
                    start=(kt == 0), stop=(kt == N_KT - 1),
                )
                if kt == 0:
                    nc.vector.tensor_copy(z_acc[:], e_t[:])
                else:
                    nc.vector.tensor_add(z_acc[:], z_acc[:], e_t[:])

            def attn_head(qc, h, filler=None, prev_tail=None):
                q0 = qc * 512
                pv = cur_pools["ppv"].tile([128, 512], F32, name=f"pv{qc}_{h}", tag=f"pv{h % 2}")
                z_acc = zpool.tile([128, 512], F32, name=f"z{qc}_{h}", tag="zacc")
                stps = {}
                for kt in range(N_KT):
                    if prev_tail is not None:
                        if kt == 2:
                            prev_tail.stage_a()
                        elif kt == 8:
                            prev_tail.stage_b()
                    stp = cur_pools["psc"].tile([128, 512], F32, name=f"st{qc}_{h}_{kt}", tag="st")
                    stps[kt] = stp
                    nc.tensor.matmul(
                        stp[:],
                        kt_sb[:, kt * 128:(kt + 1) * 128],
                        qt_sb[:, h, q0:q0 + 512],
                        start=True, stop=True,
                    )
                    run_filler(filler)
                    if kt > 0:
                        emit_ev(qc, h, kt - 1, stps, pv, z_acc)
                emit_ev(qc, h, N_KT - 1, stps, pv, z_acc)
                run_filler(filler)
                return NormTail(qc, h, pv, z_acc)

            # ================= phase 1: projections chunks 0-2 =============
            with tc.tile_pool(name="wqp", bufs=1) as wqp:
                wq_sb = wqp.tile([128, N_KC, 512], BF)
                for ch in range(4):
                    nc.sync.dma_start(
                        out=wq_sb[:, ch * 8:(ch + 1) * 8, :],
                        in_=wqt[:, ch * 8:(ch + 1) * 8, :],
                    )
                with tc.tile_pool(name="pprojA", bufs=4, space="PSUM") as pprojA:
                    pprojA._ropetag = "pacc"
                    for c in range(3):
                        scope = nc.named_scope(f"proj{c}"); scope.__enter__()
                        load_xt_group(c + 1, 0)
                        proj_sweep(pprojA, pprojA, c, "k")
                        load_xt_group(c + 1, 1)
                        proj_sweep(pprojA, pprojA, c, "v")
                        load_xt_group(c + 1, 2)
                        load_xt_group(c + 1, 3)
                        for hh in range(N_QH):
                            proj_sweep(pprojA, pprojA, c, "q", hh)
                            load_xt_group(c + 1, 4 + hh)
                        scope.__exit__(None, None, None)

                # ============ phase 2: c3 K/V + attention chunk 0 ==========
                # (pprojA closed; fresh 8-bank layout: psc 3 + ppv 2 + pzb 1
                #  + pprojB 2)
                with (
                    tc.tile_pool(name="pscA", bufs=3, space="PSUM") as psc,
                    tc.tile_pool(name="ppvA", bufs=1, space="PSUM") as ppv,
                    tc.tile_pool(name="pzbA", bufs=1, space="PSUM") as pzb,
                    tc.tile_pool(name="pprojB", bufs=2, space="PSUM") as pprojB,
                ):
                    pzb._ropetag = "zb"
                    cur_pools["psc"], cur_pools["ppv"], cur_pools["pzb"] = (
                        psc, ppv, pzb
                    )
                    scope = nc.named_scope("proj3"); scope.__enter__()
                    proj_sweep(pprojB, pzb, 3, "k")
                    proj_sweep(pprojB, pzb, 3, "v")
                    scope.__exit__(None, None, None)

                    scope = nc.named_scope("attn0"); scope.__enter__()
                    tail = None
                    for h in range(N_QH):
                        g = proj_sweep_gen(pprojB, pzb, 3, "q", h)
                        tail = attn_head(0, h, filler=g, prev_tail=tail)
                    # flush the last head's tail inside this pool scope
                    tail.stage_a()
                    tail.stage_b()
                    tail = None
                    flush_rope(pzb)
                    load_rhs_halves(0, 0)
                    scope.__exit__(None, None, None)

            # ======== phase 3: attention chunks 1-3 + wo ===========
            with (
                tc.tile_pool(name="wop", bufs=1) as wop_s,
                tc.tile_pool(name="pscB", bufs=3, space="PSUM") as psc,
                tc.tile_pool(name="ppvB", bufs=1, space="PSUM") as ppv,
                tc.tile_pool(name="pzbB", bufs=1, space="PSUM") as pzb,
                tc.tile_pool(name="pwo", bufs=1, space="PSUM") as pwo,
            ):
                cur_pools["psc"], cur_pools["ppv"], cur_pools["pzb"] = (
                    psc, ppv, pzb
                )
                wo_sb = wop_s.tile([128, N_KC, 512], BF)
                for ch in range(4):
                    nc.sync.dma_start(
                        out=wo_sb[:, ch * 8:(ch + 1) * 8, :],
                        in_=wot[:, ch * 8:(ch + 1) * 8, :],
                    )
                pending = [None]
                for qc in range(1, N_TC):
                    scope = nc.named_scope(f"attn{qc}"); scope.__enter__()
                    if qc >= 2:
                        pending.append(wo_quarter_gen(qc - 2, pwo, 3))
                    for q in range(3):
                        pending.append(wo_quarter_gen(qc - 1, pwo, q))
                    for h in range(N_QH):
                        f = pending.pop(0) if pending else None
                        tail = attn_head(qc, h, filler=f, prev_tail=tail)
                        run_filler(f, 16)
                    # wave0 halves of this chunk's gather (launched
                    # mid-block) load during the next block's head 0
                    if qc < 3:
                        load_rhs_halves(qc, 0)
                    scope.__exit__(None, None, None)
                scope = nc.named_scope("wo3"); scope.__enter__()
                tail.stage_a()
                tail.stage_b()
                pending.append(wo_quarter_gen(N_TC - 2, pwo, 3))
                for f in pending:   # quarter 3 of chunk 2
                    run_filler(f, 17)
                wo3_pass(pwo, 0)
                wo3_pass(pwo, 1)
                scope.__exit__(None, None, None)

    _split_multi_waits(nc)
    return nc


def _host_prep(x, cos, sin, wq, wk, wv, wo):
    scale = np.float32(HD ** -0.5)
    perm = np.concatenate([np.arange(0, HD, 2), np.arange(1, HD, 2)])

    xt = np.ascontiguousarray(x.T.reshape(N_KC, 128, S)).astype(_BF16)
    cosT = cos.T.astype(np.float32)
    sinT = sin.T.astype(np.float32)
    cs1 = np.concatenate([cosT, sinT], axis=0).astype(_BF16)
    cs2 = np.concatenate([sinT, cosT], axis=0).astype(_BF16)

    m1 = np.zeros((HD, HD), np.float32)
    m1[np.arange(64), np.arange(64)] = 1.0
    m1[np.arange(64) + 64, np.arange(64)] = -1.0
    m2 = np.zeros((HD, HD), np.float32)
    m2[np.arange(64), np.arange(64) + 64] = 1.0
    m2[np.arange(64) + 64, np.arange(64) + 64] = 1.0

    def to_tiles(wT, ncols):
        return np.ascontiguousarray(
            wT.reshape(N_KC, 128, ncols).transpose(1, 0, 2)
        ).astype(_BF16)

    shared = {
        "xt": xt,
        "cs1": cs1,
        "cs2": cs2,
        "mix1": m1.astype(_BF16),
        "mix2": m2.astype(_BF16),
        "onesc": np.ones((HD, 1), np.float32),
        "onesr": np.ones((1, HD), np.float32).astype(_BF16),
    }
    in_maps = []
    for c in range(N_CORES):
        wq_c = wq[c * 512:(c + 1) * 512].reshape(N_QH, HD, D)[:, perm, :]
        wq_c = wq_c.reshape(512, D) * scale
        wk_c = wk[c * HD:(c + 1) * HD][perm, :]
        wv_c = wv[c * HD:(c + 1) * HD]
        wo_c = wo[c * 512:(c + 1) * 512]
        m = dict(shared)
        m["wqt"] = to_tiles(np.ascontiguousarray(wq_c.T), 512)
        m["wkt"] = to_tiles(np.ascontiguousarray(wk_c.T), HD)
        m["wvt"] = to_tiles(np.ascontiguousarray(wv_c.T), HD)
        m["wot"] = to_tiles(np.ascontiguousarray(wo_c.T), 512)
        in_maps.append(m)
    return in_maps


def kernel(x, cos, sin, wq, wk, wv, wo, _trace=False):
    x = np.asarray(x, np.float32)
    cos = np.asarray(cos, np.float32)
    sin = np.asarray(sin, np.float32)
    wq = np.asarray(wq, np.float32)
    wk = np.asarray(wk, np.float32)
    wv = np.asarray(wv, np.float32)
    wo = np.asarray(wo, np.float32)

    in_maps = _host_prep(x, cos, sin, wq, wk, wv, wo)
    if "nc" not in _NC_CACHE:
        _NC_CACHE["nc"] = _build()
    nc = _NC_CACHE["nc"]
    res = run_bass_kernel_spmd(
        nc, in_maps, core_ids=list(range(N_CORES)), trace=_trace
    )
    out = np.concatenate([res.results[c]["out"] for c in range(N_CORES)], axis=1)
    out = np.ascontiguousarray(out, dtype=np.float32)
    if _trace:
        kernel._last_exec_time_ns = res.exec_time_ns
        kernel._last_result = res
    return out


# revision 30
# speedup vs baseline: 1.1076x; 1.1076x over previous
"""GQA attention (S=2048, D=4096, 32 Q heads / 8 KV heads, RoPE, full attn)
distributed over 8 Trainium2 NeuronCores.

Strategy (tensor-parallel by heads; AllGather of normalized attention before
the output projection):
  - core c owns Q heads 4c..4c+3 and KV head c (GQA groups align with cores).
  - all GEMMs bf16 (stationary and moving), f32 PSUM accumulation.
  - projections as transposed GEMMs QT/KT/VT [chan, tok]: sweeps PAIRED so
    consecutive matmuls alternate between two PSUM banks (single-bank
    back-to-back accumulation loses ~70ns/MM of drain overlap); chunks 0-2
    use a dedicated 4-bank ring, chunk 3's K/V + Q sweeps share a 2-bank
    ring with the attention-chunk-0 fillers.
  - RoPE via DVE muls (u=p*cs1, v=p*cs2) + deferred PE mix-matmuls flushed
    mid-next-sweep (never blocks the PE on the DVE); V transposed by
    SBUF->SBUF transposing DMAs.
  - attention per 512-token q-chunk, per head: scores ST=[k,q] per k-tile
    (3-bank ring), exp on ScalarE -> bf16, z on DVE (running f32 sum +
    ones-matmul partition reduce), PV accumulated over 16 k-tiles;
    normalize (1/z via ScalarE ln/exp, ones-broadcast matmul, DVE mul)
    deferred into the NEXT head's kt stream so the PE never waits.
  - the attention inner loop is exp-paced, so independent filler matmuls
    (wo-GEMM quarters of earlier chunks / last chunk's Q sweeps) fill the
    in-order PE queue between score/PV pairs.
  - AllGather per (chunk, head-pair) for chunks 0-2 ([256,512]bf16 ->
    [2048,512]) and per HEAD for chunk 3 ([128,512] -> [1024,512]) so the
    tail wo GEMM never waits; gathered tiles are DMA'd in half-waves
    pre-issued as soon as each collective is launched.
Host side only reshapes/transposes/casts inputs and concatenates outputs.
"""
import sys

import numpy as np
import ml_dtypes

_BF16 = ml_dtypes.bfloat16

for _p in ("/root/.axon_site/_ro/trn_rl_repo", "/opt/trn_rl_repo"):
    if _p not in sys.path:
        sys.path.append(_p)

import concourse.bass as bass
import concourse.tile as tile
from concourse import mybir
from concourse.bass_utils import run_bass_kernel_spmd

N_CORES = 8
S = 2048
D = 4096
HD = 128
N_QH = 4          # Q heads per core
N_KT = S // 128   # 16 k-tiles
N_TC = S // 512   # 4 token chunks
N_KC = D // 128   # 32 contraction tiles
F32 = mybir.dt.float32
BF = mybir.dt.bfloat16

_NC_CACHE = {}


def _split_multi_waits(nc):
    """This container's walrus accepts only ONE sync-wait per instruction
    encoding; hoist extra waits onto fresh single-wait NoOps placed before
    the instruction on the same engine."""
    n = 0
    for fn in nc.m.functions:
        for bb in fn.blocks:
            new_insts = []
            changed = False
            for ins in bb.instructions:
                si = ins.sync_info
                waits = list(si.on_wait) if si is not None else []
                if len(waits) > 1:
                    for w in waits[:-1]:
                        n += 1
                        nop = mybir.InstNoOp(name=f"WSPL-{n}", ins=[], outs=[])
                        nop.engine = ins.engine
                        nop.sync_info = mybir.SyncInfo(on_wait=[w], on_update=[])
                        new_insts.append(nop)
                    si.on_wait = waits[-1:]
                    changed = True
                new_insts.append(ins)
            if changed:
                bb.instructions = new_insts
    return n


def _build():
    nc = bass.Bass()

    xt = nc.dram_tensor("xt", [N_KC, 128, S], BF, kind="ExternalInput")
    wqt = nc.dram_tensor("wqt", [128, N_KC, 512], BF, kind="ExternalInput")
    wkt = nc.dram_tensor("wkt", [128, N_KC, HD], BF, kind="ExternalInput")
    wvt = nc.dram_tensor("wvt", [128, N_KC, HD], BF, kind="ExternalInput")
    wot = nc.dram_tensor("wot", [128, N_KC, 512], BF, kind="ExternalInput")
    cs1 = nc.dram_tensor("cs1", [HD, S], BF, kind="ExternalInput")
    cs2 = nc.dram_tensor("cs2", [HD, S], BF, kind="ExternalInput")
    mix1 = nc.dram_tensor("mix1", [HD, HD], BF, kind="ExternalInput")
    mix2 = nc.dram_tensor("mix2", [HD, HD], BF, kind="ExternalInput")
    onesc = nc.dram_tensor("onesc", [HD, 1], F32, kind="ExternalInput")
    onesr = nc.dram_tensor("onesr", [1, HD], BF, kind="ExternalInput")
    out_ext = nc.dram_tensor("out", [S, 512], F32, kind="ExternalOutput")

    # chunks 0-2: AllGather per head-pair; chunk 3: per head
    ag_in = {}
    ag_out = {}
    for qc in range(3):
        for hp in range(2):
            ag_in[(qc, hp)] = nc.dram_tensor(f"agi{qc}_{hp}", [256, 512], BF)
            ag_out[(qc, hp)] = nc.dram_tensor(
                f"ago{qc}_{hp}", [2048, 512], BF, addr_space="Shared"
            )
    for h in range(N_QH):
        ag_in[(3, h)] = nc.dram_tensor(f"agi3_{h}", [128, 512], BF)
        ag_out[(3, h)] = nc.dram_tensor(
            f"ago3_{h}", [1024, 512], BF, addr_space="Shared"
        )

    with tile.TileContext(nc) as tc:
        with (
            tc.tile_pool(name="const", bufs=1) as constp,
            tc.tile_pool(name="persist", bufs=1) as persist,
            tc.tile_pool(name="xtp", bufs=12) as xtp,
            tc.tile_pool(name="uv", bufs=2) as uvp,
            tc.tile_pool(name="vt", bufs=2) as vtp,
            tc.tile_pool(name="ep", bufs=4) as ep,
            tc.tile_pool(name="zp", bufs=2) as zpool,
            tc.tile_pool(name="small", bufs=4) as smallp,
            tc.tile_pool(name="at", bufs=3) as atp,
            tc.tile_pool(name="rhs", bufs=5) as rhsp,
            tc.tile_pool(name="fout", bufs=2) as foutp,
        ):
            # ---- constants ----
            cs1_sb = constp.tile([HD, S], BF)
            cs2_sb = constp.tile([HD, S], BF)
            mix1_sb = constp.tile([HD, HD], BF)
            mix2_sb = constp.tile([HD, HD], BF)
            onesc_sb = constp.tile([HD, 1], F32)
            onesr_sb = constp.tile([1, HD], BF)
            nc.gpsimd.dma_start(out=cs1_sb[:], in_=cs1[:])
            nc.gpsimd.dma_start(out=cs2_sb[:], in_=cs2[:])
            nc.gpsimd.dma_start(out=mix1_sb[:], in_=mix1[:])
            nc.gpsimd.dma_start(out=mix2_sb[:], in_=mix2[:])
            nc.gpsimd.dma_start(out=onesc_sb[:], in_=onesc[:])
            nc.gpsimd.dma_start(out=onesr_sb[:], in_=onesr[:])

            # ---- weights (wq and wo phases don't overlap: scoped pools) ----
            wk_sb = persist.tile([128, N_KC, HD], BF)
            wv_sb = persist.tile([128, N_KC, HD], BF)
            nc.sync.dma_start(out=wk_sb[:], in_=wkt[:])
            nc.sync.dma_start(out=wv_sb[:], in_=wvt[:])

            # ---- persistent activations ----
            qt_sb = persist.tile([128, N_QH, S], BF)
            kt_sb = persist.tile([128, S], BF)
            v_sb = persist.tile([128, N_KT, HD], BF)

            xt_tiles = {}   # (c, g) -> sbuf tile [128, 4, 512]

            def load_xt_group(c, g):
                t = xtp.tile([128, 4, 512], BF, name=f"xt{c}_{g}", tag="xt")
                nc.sync.dma_start(
                    out=t[:],
                    in_=xt[g * 4:(g + 1) * 4, :, c * 512:(c + 1) * 512].rearrange(
                        "g p n -> p g n"
                    ),
                )
                xt_tiles[(c, g)] = t

            for g in range(8):
                load_xt_group(0, g)

            # rope stage 2 (PE mix matmuls + copy) is deferred and flushed
            # mid-next-sweep, so the PE never waits on the DVE muls.
            pending_rope = []

            def flush_rope(pool):
                while pending_rope:
                    u, v, dst, key = pending_rope.pop(0)
                    rps = pool.tile([128, 512], F32, name=f"rps{key}", tag=pool._ropetag)
                    nc.tensor.matmul(rps[:], mix1_sb[:], u[:], start=True, stop=False)
                    nc.tensor.matmul(rps[:], mix2_sb[:], v[:], start=False, stop=True)
                    nc.scalar.copy(dst, rps[:])

            def rope_stage1(acc, dst, t0, key):
                u = uvp.tile([128, 512], BF, name=f"u{key}", tag="u")
                v = uvp.tile([128, 512], BF, name=f"v{key}", tag="v")
                nc.vector.tensor_mul(u[:], acc[:], cs1_sb[:, t0:t0 + 512])
                nc.vector.tensor_mul(v[:], acc[:], cs2_sb[:, t0:t0 + 512])
                pending_rope.append((u, v, dst, key))

            def proj_drain(acc, c, kind, h):
                t0 = c * 512
                if kind == "q":
                    rope_stage1(acc, qt_sb[:, h, t0:t0 + 512], t0, f"q{c}_{h}")
                elif kind == "k":
                    rope_stage1(acc, kt_sb[:, t0:t0 + 512], t0, f"k{c}")
                else:
                    vt_t = vtp.tile([128, 512], BF, name=f"vt{c}", tag="vt")
                    nc.scalar.copy(vt_t[:], acc[:])
                    for g in range(4):
                        nc.sync.dma_start_transpose(
                            out=v_sb[:, c * 4 + g, :],
                            in_=vt_t[:, g * 128:(g + 1) * 128],
                        )

            def wslice_fn(kind, h):
                if kind == "q":
                    return lambda kc: wq_sb[:, kc, h * 128:(h + 1) * 128]
                if kind == "k":
                    return lambda kc: wk_sb[:, kc, :]
                return lambda kc: wv_sb[:, kc, :]

            def proj_sweep(pp, rope_pool, c, kind, h=0):
                """One 32-MM projection sweep into one PSUM bank."""
                acc = pp.tile([128, 512], F32, name=f"acc_{kind}{c}_{h}", tag="pacc")
                ws = wslice_fn(kind, h)
                for kc in range(N_KC):
                    if kc == 8:
                        flush_rope(rope_pool)
                    nc.tensor.matmul(
                        acc[:], ws(kc), xt_tiles[(c, kc // 4)][:, kc % 4, :],
                        start=(kc == 0), stop=(kc == N_KC - 1),
                    )
                proj_drain(acc, c, kind, h)

            def proj_sweep_gen(pp, rope_pool, c, kind, h=0):
                """proj_sweep as a filler generator: 16 yields of 2 MMs."""
                acc = pp.tile([128, 512], F32, name=f"acc_{kind}{c}_{h}", tag="pacc")
                ws = wslice_fn(kind, h)
                for kp in range(16):
                    if kp == 4:
                        flush_rope(rope_pool)
                    for kc in (2 * kp, 2 * kp + 1):
                        nc.tensor.matmul(
                            acc[:], ws(kc), xt_tiles[(c, kc // 4)][:, kc % 4, :],
                            start=(kc == 0), stop=(kc == N_KC - 1),
                        )
                    if kp < 15:
                        yield
                proj_drain(acc, c, kind, h)
                yield

            # ---------- wo GEMM machinery ----------
            wo_state = {}
            cur_pools = {}   # phase-scoped PSUM pools for attention

            def load_rhs_halves(qc, hp):
                """DMA one gathered wave into two [128,8,512] half tiles."""
                halves = []
                for half in range(2):
                    r = rhsp.tile(
                        [128, 8, 512], BF, name=f"rhs{qc}_{hp}_{half}", tag="rhs"
                    )
                    nc.sync.dma_start(
                        out=r[:],
                        in_=ag_out[(qc, hp)][half * 1024:(half + 1) * 1024, :]
                        .rearrange("(t p) n -> p t n", p=128),
                    )
                    halves.append(r)
                wo_state[(qc, hp)] = halves

            def load_rhs3(h):
                r = rhsp.tile([128, 8, 512], BF, name=f"rhs3_{h}", tag="rhs")
                nc.sync.dma_start(
                    out=r[:],
                    in_=ag_out[(3, h)].rearrange("(t p) n -> p t n", p=128),
                )
                wo_state[(3, h)] = r

            def wo_quarter_gen(qc, wop, quarter):
                """wo GEMM for chunk qc (0..2), one quarter: 16 yields x 2 MMs.
                quarter 0: fps01 += wave0      quarter 1: fps01 += wave1, fout
                quarter 2: fps23 += wave0      quarter 3: fps23 += wave1, fout"""
                q0 = qc * 512
                hp = quarter % 2
                qsp = quarter // 2
                halves = wo_state[(qc, hp)]
                if hp == 0:
                    fps = [
                        wop.tile(
                            [128, 512], F32, name=f"f{qc}_{qsp}_{j}", tag=f"f{j}"
                        )
                        for j in range(2)
                    ]
                    wo_state[(qc, "fps", qsp)] = fps
                fps = wo_state[(qc, "fps", qsp)]
                for i in range(16):
                    ci, jj = i // 2, i % 2
                    hk = 4 * ci + 2 * hp + jj
                    rhs = halves[0] if ci < 4 else halves[1]
                    li = (ci % 4) * 2 + jj
                    for j in range(2):
                        qs = qsp * 2 + j
                        nc.tensor.matmul(
                            fps[j][:],
                            rhs[:, li, qs * 128:(qs + 1) * 128],
                            wo_sb[:, hk, :],
                            start=(hp == 0 and i == 0),
                            stop=(hp == 1 and i == 15),
                        )
                    if i < 15:
                        yield
                if hp == 1:
                    for j in range(2):
                        qs = qsp * 2 + j
                        f_sb = foutp.tile(
                            [128, 512], F32, name=f"fs{qc}_{qsp}_{j}", tag="fs"
                        )
                        nc.scalar.copy(f_sb[:], fps[j][:])
                        nc.sync.dma_start(
                            out=out_ext[q0 + qs * 128:q0 + (qs + 1) * 128, :],
                            in_=f_sb[:],
                        )
                yield

            def wo3_pass(wop, qsp):
                """Last chunk: one fps pair accumulated across 4 head-waves."""
                q0 = 3 * 512
                fps = [
                    wop.tile([128, 512], F32, name=f"f3_{qsp}_{j}", tag=f"f{j}")
                    for j in range(2)
                ]
                for h in range(N_QH):
                    rhs = wo_state[(3, h)]
                    for ci in range(8):
                        hk = 4 * ci + h
                        for j in range(2):
                            qs = qsp * 2 + j
                            nc.tensor.matmul(
                                fps[j][:],
                                rhs[:, ci, qs * 128:(qs + 1) * 128],
                                wo_sb[:, hk, :],
                                start=(h == 0 and ci == 0),
                                stop=(h == N_QH - 1 and ci == 7),
                            )
                for j in range(2):
                    qs = qsp * 2 + j
                    f_sb = foutp.tile(
                        [128, 512], F32, name=f"fs3_{qsp}_{j}", tag="fs"
                    )
                    nc.scalar.copy(f_sb[:], fps[j][:])
                    nc.sync.dma_start(
                        out=out_ext[q0 + qs * 128:q0 + (qs + 1) * 128, :],
                        in_=f_sb[:],
                    )

            def run_filler(f, n=1):
                if f is None:
                    return
                for _ in range(n):
                    try:
                        next(f)
                    except StopIteration:
                        break

            def ag_launch(qc, part):
                nc.gpsimd.collective_compute(
                    "AllGather",
                    mybir.AluOpType.bypass,
                    replica_groups=[list(range(N_CORES))],
                    ins=[ag_in[(qc, part)][:].opt()],
                    outs=[ag_out[(qc, part)][:].opt()],
                )

            class NormTail:
                """Deferred per-head softmax normalization, emitted inside the
                NEXT head's kt stream (kt2: z-reduce + 1/z; kt8: broadcast,
                normalize, store, collective launch + gathered-wave DMA)."""

                def __init__(self, qc, h, pv, z_acc):
                    self.qc, self.h, self.pv, self.z_acc = qc, h, pv, z_acc

                def stage_a(self):
                    qc, h = self.qc, self.h
                    zr = cur_pools["pzb"].tile([1, 512], F32, name=f"zr{qc}_{h}", tag="zb")
                    nc.tensor.matmul(
                        zr[:], onesc_sb[:], self.z_acc[:], start=True, stop=True
                    )
                    lnz = smallp.tile([1, 512], F32, name=f"ln{qc}_{h}", tag="lnz")
                    nc.scalar.activation(
                        out=lnz[:], in_=zr[:],
                        func=mybir.ActivationFunctionType.Ln,
                    )
                    self.invz = smallp.tile([1, 512], BF, name=f"iz{qc}_{h}", tag="iz")
                    nc.scalar.activation(
                        out=self.invz[:], in_=lnz[:],
                        func=mybir.ActivationFunctionType.Exp, scale=-1.0,
                    )

                def stage_b(self):
                    qc, h = self.qc, self.h
                    bc = cur_pools["pzb"].tile([128, 512], F32, name=f"bc{qc}_{h}", tag="zb")
                    nc.tensor.matmul(
                        bc[:], onesr_sb[:], self.invz[:], start=True, stop=True
                    )
                    bc_sb = smallp.tile([128, 512], BF, name=f"bcs{qc}_{h}", tag="bcs")
                    nc.scalar.copy(bc_sb[:], bc[:])
                    at_sb = atp.tile([128, 512], BF, name=f"at{qc}_{h}", tag="at")
                    nc.vector.tensor_mul(at_sb[:], self.pv[:], bc_sb[:])
                    if qc < 3:
                        hp, j = h // 2, h % 2
                        nc.gpsimd.dma_start(
                            out=ag_in[(qc, hp)][j * 128:(j + 1) * 128, :],
                            in_=at_sb[:],
                        )
                        if j == 1:
                            ag_launch(qc, hp)
                            if hp == 1:
                                # wave1 halves: ring slot is free by now
                                load_rhs_halves(qc, 1)
                    else:
                        nc.gpsimd.dma_start(out=ag_in[(3, h)][:], in_=at_sb[:])
                        ag_launch(3, h)
                        load_rhs3(h)

            def emit_ev(qc, h, kt, stps, pv, z_acc):
                e_t = ep.tile([128, 512], BF, name=f"e{qc}_{h}_{kt}", tag="e")
                nc.scalar.activation(
                    out=e_t[:], in_=stps[kt][:],
                    func=mybir.ActivationFunctionType.Exp,
                )
                nc.tensor.matmul(
                    pv[:], v_sb[:, kt, :], e_t[:],
                    start=(kt == 0), stop=(kt == N_KT - 1),
                )
                if kt == 0:
                    nc.vector.tensor_copy(z_acc[:], e_t[:])
                else:
                    nc.vector.tensor_add(z_acc[:], z_acc[:], e_t[:])

            def attn_head(qc, h, filler=None, prev_tail=None):
                q0 = qc * 512
                pv = cur_pools["ppv"].tile([128, 512], F32, name=f"pv{qc}_{h}", tag=f"pv{h % 2}")
                z_acc = zpool.tile([128, 512], F32, name=f"z{qc}_{h}", tag="zacc")
                stps = {}
                for kt in range(N_KT):
                    if prev_tail is not None:
                        if kt == 2:
                            prev_tail.stage_a()
                        elif kt == 8:
                            prev_tail.stage_b()
                    stp = cur_pools["psc"].tile([128, 512], F32, name=f"st{qc}_{h}_{kt}", tag="st")
                    stps[kt] = stp
                    nc.tensor.matmul(
                        stp[:],
                        kt_sb[:, kt * 128:(kt + 1) * 128],
                        qt_sb[:, h, q0:q0 + 512],
                        start=True, stop=True,
                    )
                    run_filler(filler)
                    if kt > 0:
                        emit_ev(qc, h, kt - 1, stps, pv, z_acc)
                emit_ev(qc, h, N_KT - 1, stps, pv, z_acc)
                run_filler(filler)
                return NormTail(qc, h, pv, z_acc)

            # ================= phase 1: projections chunks 0-2 =============
            with tc.tile_pool(name="wqp", bufs=1) as wqp:
                wq_sb = wqp.tile([128, N_KC, 512], BF)
                for ch in range(4):
                    nc.sync.dma_start(
                        out=wq_sb[:, ch * 8:(ch + 1) * 8, :],
                        in_=wqt[:, ch * 8:(ch + 1) * 8, :],
                    )
                with tc.tile_pool(name="pprojA", bufs=4, space="PSUM") as pprojA:
                    pprojA._ropetag = "pacc"
                    for c in range(3):
                        scope = nc.named_scope(f"proj{c}"); scope.__enter__()
                        load_xt_group(c + 1, 0)
                        proj_sweep(pprojA, pprojA, c, "k")
                        load_xt_group(c + 1, 1)
                        proj_sweep(pprojA, pprojA, c, "v")
                        load_xt_group(c + 1, 2)
                        load_xt_group(c + 1, 3)
                        for hh in range(N_QH):
                            proj_sweep(pprojA, pprojA, c, "q", hh)
                            load_xt_group(c + 1, 4 + hh)
                        scope.__exit__(None, None, None)

                # ============ phase 2: c3 K/V + attention chunk 0 ==========
                # (pprojA closed; fresh 8-bank layout: psc 3 + ppv 2 + pzb 1
                #  + pprojB 2)
                with (
                    tc.tile_pool(name="pscA", bufs=3, space="PSUM") as psc,
                    tc.tile_pool(name="ppvA", bufs=1, space="PSUM") as ppv,
                    tc.tile_pool(name="pzbA", bufs=1, space="PSUM") as pzb,
                    tc.tile_pool(name="pprojB", bufs=2, space="PSUM") as pprojB,
                ):
                    pzb._ropetag = "zb"
                    cur_pools["psc"], cur_pools["ppv"], cur_pools["pzb"] = (
                        psc, ppv, pzb
                    )
                    scope = nc.named_scope("proj3"); scope.__enter__()
                    proj_sweep(pprojB, pzb, 3, "k")
                    proj_sweep(pprojB, pzb, 3, "v")
                    scope.__exit__(None, None, None)

                    scope = nc.named_scope("attn0"); scope.__enter__()
                    tail = None
                    for h in range(N_QH):
                        g = proj_sweep_gen(pprojB, pzb, 3, "q", h)
                        tail = attn_head(0, h, filler=g, prev_tail=tail)
                    # flush the last head's tail inside this pool scope
                    tail.stage_a()
                    tail.stage_b()
                    tail = None
                    flush_rope(pzb)
                    load_rhs_halves(0, 0)
                    scope.__exit__(None, None, None)

            # ======== phase 3: attention chunks 1-3 + wo ===========
            with (
                tc.tile_pool(name="wop", bufs=1) as wop_s,
                tc.tile_pool(name="pscB", bufs=3, space="PSUM") as psc,
                tc.tile_pool(name="ppvB", bufs=1, space="PSUM") as ppv,
                tc.tile_pool(name="pzbB", bufs=1, space="PSUM") as pzb,
                tc.tile_pool(name="pwo", bufs=1, space="PSUM") as pwo,
            ):
                cur_pools["psc"], cur_pools["ppv"], cur_pools["pzb"] = (
                    psc, ppv, pzb
                )
                wo_sb = wop_s.tile([128, N_KC, 512], BF)
                for ch in range(4):
                    nc.sync.dma_start(
                        out=wo_sb[:, ch * 8:(ch + 1) * 8, :],
                        in_=wot[:, ch * 8:(ch + 1) * 8, :],
                    )
                pending = [None, None]
                for qc in range(1, N_TC):
                    scope = nc.named_scope(f"attn{qc}"); scope.__enter__()
                    if qc >= 2:
                        pending.append(wo_quarter_gen(qc - 2, pwo, 3))
                    for q in range(3):
                        pending.append(wo_quarter_gen(qc - 1, pwo, q))
                    for h in range(N_QH):
                        f = pending.pop(0) if pending else None
                        tail = attn_head(qc, h, filler=f, prev_tail=tail)
                        run_filler(f, 16)
                    # wave0 halves of this chunk's gather (launched
                    # mid-block) load during the next block's head 0
                    if qc < 3:
                        load_rhs_halves(qc, 0)
                    scope.__exit__(None, None, None)
                scope = nc.named_scope("wo3"); scope.__enter__()
                tail.stage_a()
                tail.stage_b()
                pending.append(wo_quarter_gen(N_TC - 2, pwo, 3))
                for f in pending:   # quarter 3 of chunk 2
                    run_filler(f, 17)
                wo3_pass(pwo, 0)
                wo3_pass(pwo, 1)
                scope.__exit__(None, None, None)

    _split_multi_waits(nc)
    return nc


def _host_prep(x, cos, sin, wq, wk, wv, wo):
    scale = np.float32(HD ** -0.5)
    perm = np.concatenate([np.arange(0, HD, 2), np.arange(1, HD, 2)])

    xt = np.ascontiguousarray(x.T.reshape(N_KC, 128, S)).astype(_BF16)
    cosT = cos.T.astype(np.float32)
    sinT = sin.T.astype(np.float32)
    cs1 = np.concatenate([cosT, sinT], axis=0).astype(_BF16)
    cs2 = np.concatenate([sinT, cosT], axis=0).astype(_BF16)

    m1 = np.zeros((HD, HD), np.float32)
    m1[np.arange(64), np.arange(64)] = 1.0
    m1[np.arange(64) + 64, np.arange(64)] = -1.0
    m2 = np.zeros((HD, HD), np.float32)
    m2[np.arange(64), np.arange(64) + 64] = 1.0
    m2[np.arange(64) + 64, np.arange(64) + 64] = 1.0

    def to_tiles(wT, ncols):
        return np.ascontiguousarray(
            wT.reshape(N_KC, 128, ncols).transpose(1, 0, 2)
        ).astype(_BF16)

    shared = {
        "xt": xt,
        "cs1": cs1,
        "cs2": cs2,
        "mix1": m1.astype(_BF16),
        "mix2": m2.astype(_BF16),
        "onesc": np.ones((HD, 1), np.float32),
        "onesr": np.ones((1, HD), np.float32).astype(_BF16),
    }
    in_maps = []
    for c in range(N_CORES):
        wq_c = wq[c * 512:(c + 1) * 512].reshape(N_QH, HD, D)[:, perm, :]
        wq_c = wq_c.reshape(512, D) * scale
        wk_c = wk[c * HD:(c + 1) * HD][perm, :]
        wv_c = wv[c * HD:(c + 1) * HD]
        wo_c = wo[c * 512:(c + 1) * 512]
        m = dict(shared)
        m["wqt"] = to_tiles(np.ascontiguousarray(wq_c.T), 512)
        m["wkt"] = to_tiles(np.ascontiguousarray(wk_c.T), HD)
        m["wvt"] = to_tiles(np.ascontiguousarray(wv_c.T), HD)
        m["wot"] = to_tiles(np.ascontiguousarray(wo_c.T), 512)
        in_maps.append(m)
    return in_maps


def kernel(x, cos, sin, wq, wk, wv, wo, _trace=False):
    x = np.asarray(x, np.float32)
    cos = np.asarray(cos, np.float32)
    sin = np.asarray(sin, np.float32)
    wq = np.asarray(wq, np.float32)
    wk = np.asarray(wk, np.float32)
    wv = np.asarray(wv, np.float32)
    wo = np.asarray(wo, np.float32)

    in_maps = _host_prep(x, cos, sin, wq, wk, wv, wo)
    if "nc" not in _NC_CACHE:
        _NC_CACHE["nc"] = _build()
    nc = _NC_CACHE["nc"]
    res = run_bass_kernel_spmd(
        nc, in_maps, core_ids=list(range(N_CORES)), trace=_trace
    )
    out = np.concatenate([res.results[c]["out"] for c in range(N_CORES)], axis=1)
    out = np.ascontiguousarray(out, dtype=np.float32)
    if _trace:
        kernel._last_exec_time_ns = res.exec_time_ns
        kernel._last_result = res
    return out


# revision 36
# speedup vs baseline: 1.1096x; 1.0018x over previous
"""GQA attention (S=2048, D=4096, 32 Q heads / 8 KV heads, RoPE, full attn)
distributed over 8 Trainium2 NeuronCores.

Strategy (tensor-parallel by heads; AllGather of normalized attention before
the output projection):
  - core c owns Q heads 4c..4c+3 and KV head c (GQA groups align with cores).
  - all GEMMs bf16 (stationary and moving), f32 PSUM accumulation.
  - projections as transposed GEMMs QT/KT/VT [chan, tok]: staggered 32-MM
    single-bank sweeps (one PSUM bank per output; banks release one at a
    time so RoPE drains overlap the next sweep); chunks 0-2 use a 4-bank
    ring, chunk 3's K/V + Q sweeps share a 2-bank ring with the
    attention-chunk-0 fillers.
  - RoPE via DVE muls (u=p*cs1, v=p*cs2) + deferred PE mix-matmuls flushed
    mid-next-sweep (never blocks the PE on the DVE); V transposed by
    SBUF->SBUF transposing DMAs.
  - attention per 512-token q-chunk, per head: scores ST=[k,q] per k-tile
    (3-bank ring), exp on ScalarE -> bf16, z on DVE (running f32 sum +
    ones-matmul partition reduce), PV accumulated over 16 k-tiles;
    normalize (1/z via ScalarE ln/exp, ones-broadcast matmul, DVE mul)
    deferred into the NEXT head's kt stream so the PE never waits.
  - the attention inner loop is exp-paced, so independent filler matmuls
    (wo-GEMM quarters of earlier chunks / last chunk's Q sweeps) fill the
    in-order PE queue between score/PV pairs.
  - AllGather per (chunk, head-pair) for chunks 0-2 ([256,512]bf16 ->
    [2048,512]) and per HEAD for chunk 3 ([128,512] -> [1024,512]) so the
    tail wo GEMM never waits; gathered tiles are DMA'd in half-waves
    pre-issued as soon as each collective is launched.
Host side only reshapes/transposes/casts inputs and concatenates outputs.
"""
import sys

import numpy as np
import ml_dtypes

_BF16 = ml_dtypes.bfloat16

for _p in ("/root/.axon_site/_ro/trn_rl_repo", "/opt/trn_rl_repo"):
    if _p not in sys.path:
        sys.path.append(_p)

import concourse.bass as bass
import concourse.tile as tile
from concourse import mybir
from concourse.bass_utils import run_bass_kernel_spmd

N_CORES = 8
S = 2048
D = 4096
HD = 128
N_QH = 4          # Q heads per core
N_KT = S // 128   # 16 k-tiles
N_TC = S // 512   # 4 token chunks
N_KC = D // 128   # 32 contraction tiles
F32 = mybir.dt.float32
BF = mybir.dt.bfloat16

_NC_CACHE = {}


def _split_multi_waits(nc):
    """This container's walrus accepts only ONE sync-wait per instruction
    encoding; hoist extra waits onto fresh single-wait NoOps placed before
    the instruction on the same engine."""
    n = 0
    for fn in nc.m.functions:
        for bb in fn.blocks:
            new_insts = []
            changed = False
            for ins in bb.instructions:
                si = ins.sync_info
                waits = list(si.on_wait) if si is not None else []
                if len(waits) > 1:
                    for w in waits[:-1]:
                        n += 1
                        nop = mybir.InstNoOp(name=f"WSPL-{n}", ins=[], outs=[])
                        nop.engine = ins.engine
                        nop.sync_info = mybir.SyncInfo(on_wait=[w], on_update=[])
                        new_insts.append(nop)
                    si.on_wait = waits[-1:]
                    changed = True
                new_insts.append(ins)
            if changed:
                bb.instructions = new_insts
    return n


def _build():
    nc = bass.Bass()

    xt = nc.dram_tensor("xt", [N_KC, 128, S], BF, kind="ExternalInput")
    wqt = nc.dram_tensor("wqt", [128, N_KC, 512], BF, kind="ExternalInput")
    wkt = nc.dram_tensor("wkt", [128, N_KC, HD], BF, kind="ExternalInput")
    wvt = nc.dram_tensor("wvt", [128, N_KC, HD], BF, kind="ExternalInput")
    wot = nc.dram_tensor("wot", [128, N_KC, 512], BF, kind="ExternalInput")
    cs1 = nc.dram_tensor("cs1", [HD, S], BF, kind="ExternalInput")
    cs2 = nc.dram_tensor("cs2", [HD, S], BF, kind="ExternalInput")
    mix1 = nc.dram_tensor("mix1", [HD, HD], BF, kind="ExternalInput")
    mix2 = nc.dram_tensor("mix2", [HD, HD], BF, kind="ExternalInput")
    onesc = nc.dram_tensor("onesc", [HD, 1], F32, kind="ExternalInput")
    onesr = nc.dram_tensor("onesr", [1, HD], BF, kind="ExternalInput")
    out_ext = nc.dram_tensor("out", [S, 512], F32, kind="ExternalOutput")

    # chunks 0-2: AllGather per head-pair; chunk 3: per head
    ag_in = {}
    ag_out = {}
    for qc in range(3):
        for hp in range(2):
            ag_in[(qc, hp)] = nc.dram_tensor(f"agi{qc}_{hp}", [256, 512], BF)
            ag_out[(qc, hp)] = nc.dram_tensor(
                f"ago{qc}_{hp}", [2048, 512], BF, addr_space="Shared"
            )
    for h in range(N_QH):
        ag_in[(3, h)] = nc.dram_tensor(f"agi3_{h}", [128, 512], BF)
        ag_out[(3, h)] = nc.dram_tensor(
            f"ago3_{h}", [1024, 512], BF, addr_space="Shared"
        )

    with tile.TileContext(nc) as tc:
        with (
            tc.tile_pool(name="const", bufs=1) as constp,
            tc.tile_pool(name="persist", bufs=1) as persist,
            tc.tile_pool(name="xtp", bufs=10) as xtp,
            tc.tile_pool(name="uv", bufs=2) as uvp,
            tc.tile_pool(name="vt", bufs=2) as vtp,
            tc.tile_pool(name="ep", bufs=4) as ep,
            tc.tile_pool(name="zp", bufs=2) as zpool,
            tc.tile_pool(name="small", bufs=2) as smallp,
            tc.tile_pool(name="at", bufs=3) as atp,
            tc.tile_pool(name="rhs", bufs=7) as rhsp,
            tc.tile_pool(name="fout", bufs=2) as foutp,
        ):
            # ---- constants ----
            cs1_sb = constp.tile([HD, S], BF)
            cs2_sb = constp.tile([HD, S], BF)
            mix1_sb = constp.tile([HD, HD], BF)
            mix2_sb = constp.tile([HD, HD], BF)
            onesc_sb = constp.tile([HD, 1], F32)
            onesr_sb = constp.tile([1, HD], BF)
            nc.gpsimd.dma_start(out=cs1_sb[:], in_=cs1[:])
            nc.gpsimd.dma_start(out=cs2_sb[:], in_=cs2[:])
            nc.gpsimd.dma_start(out=mix1_sb[:], in_=mix1[:])
            nc.gpsimd.dma_start(out=mix2_sb[:], in_=mix2[:])
            nc.gpsimd.dma_start(out=onesc_sb[:], in_=onesc[:])
            nc.gpsimd.dma_start(out=onesr_sb[:], in_=onesr[:])

            # ---- persistent activations ----
            qt_sb = persist.tile([128, N_QH, S], BF)
            kt_sb = persist.tile([128, S], BF)
            v_sb = persist.tile([128, N_KT, HD], BF)

            xt_tiles = {}   # (c, g) -> sbuf tile [128, 4, 512]

            def load_xt_group(c, g):
                t = xtp.tile([128, 4, 512], BF, name=f"xt{c}_{g}", tag="xt")
                nc.sync.dma_start(
                    out=t[:],
                    in_=xt[g * 4:(g + 1) * 4, :, c * 512:(c + 1) * 512].rearrange(
                        "g p n -> p g n"
                    ),
                )
                xt_tiles[(c, g)] = t

            for g in range(8):
                load_xt_group(0, g)

            # rope stage 2 (PE mix matmuls + copy) is deferred and flushed
            # mid-next-sweep, so the PE never waits on the DVE muls.
            pending_rope = []

            def flush_rope(pool):
                while pending_rope:
                    u, v, dst, key = pending_rope.pop(0)
                    rps = pool.tile([128, 512], F32, name=f"rps{key}", tag=pool._ropetag)
                    nc.tensor.matmul(rps[:], mix1_sb[:], u[:], start=True, stop=False)
                    nc.tensor.matmul(rps[:], mix2_sb[:], v[:], start=False, stop=True)
                    nc.scalar.copy(dst, rps[:])

            def rope_stage1(acc, dst, t0, key):
                u = uvp.tile([128, 512], BF, name=f"u{key}", tag="u")
                v = uvp.tile([128, 512], BF, name=f"v{key}", tag="v")
                nc.vector.tensor_mul(u[:], acc[:], cs1_sb[:, t0:t0 + 512])
                nc.vector.tensor_mul(v[:], acc[:], cs2_sb[:, t0:t0 + 512])
                pending_rope.append((u, v, dst, key))

            def proj_drain(acc, c, kind, h):
                t0 = c * 512
                if kind == "q":
                    rope_stage1(acc, qt_sb[:, h, t0:t0 + 512], t0, f"q{c}_{h}")
                elif kind == "k":
                    rope_stage1(acc, kt_sb[:, t0:t0 + 512], t0, f"k{c}")
                else:
                    vt_t = vtp.tile([128, 512], BF, name=f"vt{c}", tag="vt")
                    nc.scalar.copy(vt_t[:], acc[:])
                    for g in range(4):
                        nc.sync.dma_start_transpose(
                            out=v_sb[:, c * 4 + g, :],
                            in_=vt_t[:, g * 128:(g + 1) * 128],
                        )

            def wslice_fn(kind, h):
                if kind == "q":
                    return lambda kc: wq_sb[:, kc, h * 128:(h + 1) * 128]
                if kind == "k":
                    return lambda kc: wk_sb[:, kc, :]
                return lambda kc: wv_sb[:, kc, :]

            def proj_sweep(pp, rope_pool, c, kind, h=0):
                """One 32-MM projection sweep into one PSUM bank."""
                acc = pp.tile([128, 512], F32, name=f"acc_{kind}{c}_{h}", tag="pacc")
                ws = wslice_fn(kind, h)
                for kc in range(N_KC):
                    if kc == 8:
                        flush_rope(rope_pool)
                    nc.tensor.matmul(
                        acc[:], ws(kc), xt_tiles[(c, kc // 4)][:, kc % 4, :],
                        start=(kc == 0), stop=(kc == N_KC - 1),
                    )
                proj_drain(acc, c, kind, h)

            def proj_sweep_gen(pp, rope_pool, c, kind, h=0, tag="pacc"):
                """proj_sweep as a filler generator: 16 yields of 2 MMs."""
                acc = pp.tile([128, 512], F32, name=f"acc_{kind}{c}_{h}", tag=tag)
                ws = wslice_fn(kind, h)
                for kp in range(16):
                    if kp == 4:
                        flush_rope(rope_pool)
                    for kc in (2 * kp, 2 * kp + 1):
                        nc.tensor.matmul(
                            acc[:], ws(kc), xt_tiles[(c, kc // 4)][:, kc % 4, :],
                            start=(kc == 0), stop=(kc == N_KC - 1),
                        )
                    if kp < 15:
                        yield
                proj_drain(acc, c, kind, h)
                yield

            # ---------- wo GEMM machinery ----------
            wo_state = {}
            cur_pools = {}   # phase-scoped PSUM pools for attention

            def load_rhs_halves(qc, hp):
                """DMA one gathered wave into two [128,8,512] half tiles."""
                halves = []
                for half in range(2):
                    r = rhsp.tile(
                        [128, 8, 512], BF, name=f"rhs{qc}_{hp}_{half}", tag="rhs"
                    )
                    nc.sync.dma_start(
                        out=r[:],
                        in_=ag_out[(qc, hp)][half * 1024:(half + 1) * 1024, :]
                        .rearrange("(t p) n -> p t n", p=128),
                    )
                    halves.append(r)
                wo_state[(qc, hp)] = halves

            def load_rhs3(h):
                r = rhsp.tile([128, 8, 512], BF, name=f"rhs3_{h}", tag="rhs")
                nc.sync.dma_start(
                    out=r[:],
                    in_=ag_out[(3, h)].rearrange("(t p) n -> p t n", p=128),
                )
                wo_state[(3, h)] = r

            def wo_quarter_gen(qc, wop, quarter):
                """wo GEMM for chunk qc (0..2), one quarter: 16 yields x 2 MMs.
                quarter 0: fps01 += wave0      quarter 1: fps01 += wave1, fout
                quarter 2: fps23 += wave0      quarter 3: fps23 += wave1, fout"""
                q0 = qc * 512
                hp = quarter % 2
                qsp = quarter // 2
                halves = wo_state[(qc, hp)]
                if hp == 0:
                    fps = [
                        wop.tile(
                            [128, 512], F32, name=f"f{qc}_{qsp}_{j}", tag=f"f{j}"
                        )
                        for j in range(2)
                    ]
                    wo_state[(qc, "fps", qsp)] = fps
                fps = wo_state[(qc, "fps", qsp)]
                for i in range(16):
                    ci, jj = i // 2, i % 2
                    hk = 4 * ci + 2 * hp + jj
                    rhs = halves[0] if ci < 4 else halves[1]
                    li = (ci % 4) * 2 + jj
                    for j in range(2):
                        qs = qsp * 2 + j
                        nc.tensor.matmul(
                            fps[j][:],
                            rhs[:, li, qs * 128:(qs + 1) * 128],
                            wo_sb[:, hk, :],
                            start=(hp == 0 and i == 0),
                            stop=(hp == 1 and i == 15),
                        )
                    if i < 15:
                        yield
                if hp == 1:
                    for j in range(2):
                        qs = qsp * 2 + j
                        f_sb = foutp.tile(
                            [128, 512], F32, name=f"fs{qc}_{qsp}_{j}", tag="fs"
                        )
                        nc.scalar.copy(f_sb[:], fps[j][:])
                        nc.sync.dma_start(
                            out=out_ext[q0 + qs * 128:q0 + (qs + 1) * 128, :],
                            in_=f_sb[:],
                        )
                yield

            def wo3_pass(wop, qsp):
                """Last chunk: one fps pair accumulated across 4 head-waves."""
                q0 = 3 * 512
                fps = [
                    wop.tile([128, 512], F32, name=f"f3_{qsp}_{j}", tag=f"f{j}")
                    for j in range(2)
                ]
                for h in range(N_QH):
                    rhs = wo_state[(3, h)]
                    for ci in range(8):
                        hk = 4 * ci + h
                        for j in range(2):
                            qs = qsp * 2 + j
                            nc.tensor.matmul(
                                fps[j][:],
                                rhs[:, ci, qs * 128:(qs + 1) * 128],
                                wo_sb[:, hk, :],
                                start=(h == 0 and ci == 0),
                                stop=(h == N_QH - 1 and ci == 7),
                            )
                for j in range(2):
                    qs = qsp * 2 + j
                    f_sb = foutp.tile(
                        [128, 512], F32, name=f"fs3_{qsp}_{j}", tag="fs"
                    )
                    nc.scalar.copy(f_sb[:], fps[j][:])
                    nc.sync.dma_start(
                        out=out_ext[q0 + qs * 128:q0 + (qs + 1) * 128, :],
                        in_=f_sb[:],
                    )

            def run_filler(f, n=1):
                if f is None:
                    return
                for _ in range(n):
                    try:
                        next(f)
                    except StopIteration:
                        break

            def ag_launch(qc, part):
                nc.gpsimd.collective_compute(
                    "AllGather",
                    mybir.AluOpType.bypass,
                    replica_groups=[list(range(N_CORES))],
                    ins=[ag_in[(qc, part)][:].opt()],
                    outs=[ag_out[(qc, part)][:].opt()],
                )

            class NormTail:
                """Deferred per-head softmax normalization, emitted inside the
                NEXT head's kt stream (kt2: z-reduce + 1/z; kt8: broadcast,
                normalize, store, collective launch + gathered-wave DMA)."""

                def __init__(self, qc, h, pv, z_acc):
                    self.qc, self.h, self.pv, self.z_acc = qc, h, pv, z_acc

                def stage_a(self):
                    qc, h = self.qc, self.h
                    zr = cur_pools["pzb"].tile([1, 512], F32, name=f"zr{qc}_{h}", tag="zb")
                    nc.tensor.matmul(
                        zr[:], onesc_sb[:], self.z_acc[:], start=True, stop=True
                    )
                    lnz = smallp.tile([1, 512], F32, name=f"ln{qc}_{h}", tag="lnz")
                    nc.scalar.activation(
                        out=lnz[:], in_=zr[:],
                        func=mybir.ActivationFunctionType.Ln,
                    )
                    self.invz = smallp.tile([1, 512], BF, name=f"iz{qc}_{h}", tag="iz")
                    nc.scalar.activation(
                        out=self.invz[:], in_=lnz[:],
                        func=mybir.ActivationFunctionType.Exp, scale=-1.0,
                    )

                def stage_b(self):
                    qc, h = self.qc, self.h
                    bc = cur_pools["pzb"].tile([128, 512], F32, name=f"bc{qc}_{h}", tag="zb")
                    nc.tensor.matmul(
                        bc[:], onesr_sb[:], self.invz[:], start=True, stop=True
                    )
                    bc_sb = smallp.tile([128, 512], BF, name=f"bcs{qc}_{h}", tag="bcs")
                    nc.scalar.copy(bc_sb[:], bc[:])
                    at_sb = atp.tile([128, 512], BF, name=f"at{qc}_{h}", tag="at")
                    nc.vector.tensor_mul(at_sb[:], self.pv[:], bc_sb[:])
                    if qc < 3:
                        hp, j = h // 2, h % 2
                        nc.gpsimd.dma_start(
                            out=ag_in[(qc, hp)][j * 128:(j + 1) * 128, :],
                            in_=at_sb[:],
                        )
                        if j == 1:
                            ag_launch(qc, hp)
                            if hp == 1:
                                # wave1 halves: ring slot is free by now
                                load_rhs_halves(qc, 1)
                    else:
                        nc.gpsimd.dma_start(out=ag_in[(3, h)][:], in_=at_sb[:])
                        ag_launch(3, h)
                        load_rhs3(h)

            def emit_ev(qc, h, kt, stps, pv, z_acc):
                e_t = ep.tile([128, 512], BF, name=f"e{qc}_{h}_{kt}", tag="e")
                nc.scalar.activation(
                    out=e_t[:], in_=stps[kt][:],
                    func=mybir.ActivationFunctionType.Exp,
                )
                nc.tensor.matmul(
                    pv[:], v_sb[:, kt, :], e_t[:],
                    start=(kt == 0), stop=(kt == N_KT - 1),
                )
                if kt == 0:
                    nc.vector.tensor_copy(z_acc[:], e_t[:])
                else:
                    nc.vector.tensor_add(z_acc[:], z_acc[:], e_t[:])

            def attn_head(qc, h, filler=None, prev_tail=None):
                q0 = qc * 512
                pv = cur_pools["ppv"].tile([128, 512], F32, name=f"pv{qc}_{h}", tag=f"pv{h % 2}")
                z_acc = zpool.tile([128, 512], F32, name=f"z{qc}_{h}", tag="zacc")
                stps = {}
                for kt in range(N_KT):
                    if prev_tail is not None:
                        if kt == 2:
                            prev_tail.stage_a()
                        elif kt == 8:
                            prev_tail.stage_b()
                    stp = cur_pools["psc"].tile([128, 512], F32, name=f"st{qc}_{h}_{kt}", tag="st")
                    stps[kt] = stp
                    nc.tensor.matmul(
                        stp[:],
                        kt_sb[:, kt * 128:(kt + 1) * 128],
                        qt_sb[:, h, q0:q0 + 512],
                        start=True, stop=True,
                    )
                    run_filler(filler)
                    if kt > 0:
                        emit_ev(qc, h, kt - 1, stps, pv, z_acc)
                emit_ev(qc, h, N_KT - 1, stps, pv, z_acc)
                run_filler(filler)
                return NormTail(qc, h, pv, z_acc)

            # ================= phase 1: projections chunks 0-2 =============
            with tc.tile_pool(name="wqp", bufs=1) as wqp:
                wk_sb = wqp.tile([128, N_KC, HD], BF)
                wv_sb = wqp.tile([128, N_KC, HD], BF)
                nc.sync.dma_start(out=wk_sb[:], in_=wkt[:])
                nc.sync.dma_start(out=wv_sb[:], in_=wvt[:])
                wq_sb = wqp.tile([128, N_KC, 512], BF)
                for ch in range(4):
                    nc.sync.dma_start(
                        out=wq_sb[:, ch * 8:(ch + 1) * 8, :],
                        in_=wqt[:, ch * 8:(ch + 1) * 8, :],
                    )
                with tc.tile_pool(name="pprojA", bufs=4, space="PSUM") as pprojA:
                    pprojA._ropetag = "pacc"
                    for c in range(3):
                        scope = nc.named_scope(f"proj{c}"); scope.__enter__()
                        load_xt_group(c + 1, 0)
                        proj_sweep(pprojA, pprojA, c, "k")
                        load_xt_group(c + 1, 1)
                        proj_sweep(pprojA, pprojA, c, "v")
                        load_xt_group(c + 1, 2)
                        load_xt_group(c + 1, 3)
                        for hh in range(N_QH):
                            proj_sweep(pprojA, pprojA, c, "q", hh)
                            load_xt_group(c + 1, 4 + hh)
                        scope.__exit__(None, None, None)

                # ============ phase 2: c3 K/V + attention chunk 0 ==========
                # (pprojA closed; fresh 8-bank layout: psc 3 + ppv 2 + pzb 1
                #  + pprojB 2)
                with (
                    tc.tile_pool(name="pscA", bufs=3, space="PSUM") as psc,
                    tc.tile_pool(name="ppvA", bufs=1, space="PSUM") as ppv,
                    tc.tile_pool(name="pzbA", bufs=1, space="PSUM") as pzb,
                    tc.tile_pool(name="pprojB", bufs=2, space="PSUM") as pprojB,
                ):
                    pzb._ropetag = "zb"
                    cur_pools["psc"], cur_pools["ppv"], cur_pools["pzb"] = (
                        psc, ppv, pzb
                    )
                    scope = nc.named_scope("proj3"); scope.__enter__()
                    proj_sweep(pprojB, pzb, 3, "k")
                    proj_sweep(pprojB, pzb, 3, "v")
                    scope.__exit__(None, None, None)

                    scope = nc.named_scope("attn0"); scope.__enter__()
                    tail = None
                    for h in range(N_QH):
                        g = proj_sweep_gen(pprojB, pzb, 3, "q", h)
                        tail = attn_head(0, h, filler=g, prev_tail=tail)
                    # flush the last head's tail inside this pool scope
                    tail.stage_a()
                    tail.stage_b()
                    tail = None
                    flush_rope(pzb)
                    load_rhs_halves(0, 0)
                    scope.__exit__(None, None, None)

            # ======== phase 3: attention chunks 1-3 + wo ===========
            with (
                tc.tile_pool(name="wop", bufs=1) as wop_s,
                tc.tile_pool(name="pscB", bufs=3, space="PSUM") as psc,
                tc.tile_pool(name="ppvB", bufs=1, space="PSUM") as ppv,
                tc.tile_pool(name="pzbB", bufs=1, space="PSUM") as pzb,
                tc.tile_pool(name="pwo", bufs=1, space="PSUM") as pwo,
            ):
                cur_pools["psc"], cur_pools["ppv"], cur_pools["pzb"] = (
                    psc, ppv, pzb
                )
                pzb._ropetag = "zb"
                wo_sb = wop_s.tile([128, N_KC, 512], BF)
                for ch in range(4):
                    nc.sync.dma_start(
                        out=wo_sb[:, ch * 8:(ch + 1) * 8, :],
                        in_=wot[:, ch * 8:(ch + 1) * 8, :],
                    )
                pending = [None, None]
                for qc in range(1, N_TC):
                    scope = nc.named_scope(f"attn{qc}"); scope.__enter__()
                    if qc >= 2:
                        pending.append(wo_quarter_gen(qc - 2, pwo, 3))
                    for q in range(3):
                        pending.append(wo_quarter_gen(qc - 1, pwo, q))
                    for h in range(N_QH):
                        f = pending.pop(0) if pending else None
                        tail = attn_head(qc, h, filler=f, prev_tail=tail)
                        run_filler(f, 16)
                    if qc < 3:
                        load_rhs_halves(qc, 0)
                    scope.__exit__(None, None, None)
                scope = nc.named_scope("wo3"); scope.__enter__()
                tail.stage_a()
                tail.stage_b()
                pending.append(wo_quarter_gen(N_TC - 2, pwo, 3))
                for f in pending:
                    run_filler(f, 17)
                wo3_pass(pwo, 0)
                wo3_pass(pwo, 1)
                scope.__exit__(None, None, None)

    _split_multi_waits(nc)
    return nc


def _host_prep(x, cos, sin, wq, wk, wv, wo):
    scale = np.float32(HD ** -0.5)
    perm = np.concatenate([np.arange(0, HD, 2), np.arange(1, HD, 2)])

    xt = np.ascontiguousarray(x.T.reshape(N_KC, 128, S)).astype(_BF16)
    cosT = cos.T.astype(np.float32)
    sinT = sin.T.astype(np.float32)
    cs1 = np.concatenate([cosT, sinT], axis=0).astype(_BF16)
    cs2 = np.concatenate([sinT, cosT], axis=0).astype(_BF16)

    m1 = np.zeros((HD, HD), np.float32)
    m1[np.arange(64), np.arange(64)] = 1.0
    m1[np.arange(64) + 64, np.arange(64)] = -1.0
    m2 = np.zeros((HD, HD), np.float32)
    m2[np.arange(64), np.arange(64) + 64] = 1.0
    m2[np.arange(64) + 64, np.arange(64) + 64] = 1.0

    def to_tiles(wT, ncols):
        return np.ascontiguousarray(
            wT.reshape(N_KC, 128, ncols).transpose(1, 0, 2)
        ).astype(_BF16)

    shared = {
        "xt": xt,
        "cs1": cs1,
        "cs2": cs2,
        "mix1": m1.astype(_BF16),
        "mix2": m2.astype(_BF16),
        "onesc": np.ones((HD, 1), np.float32),
        "onesr": np.ones((1, HD), np.float32).astype(_BF16),
    }
    in_maps = []
    for c in range(N_CORES):
        wq_c = wq[c * 512:(c + 1) * 512].reshape(N_QH, HD, D)[:, perm, :]
        wq_c = wq_c.reshape(512, D) * scale
        wk_c = wk[c * HD:(c + 1) * HD][perm, :]
        wv_c = wv[c * HD:(c + 1) * HD]
        wo_c = wo[c * 512:(c + 1) * 512]
        m = dict(shared)
        m["wqt"] = to_tiles(np.ascontiguousarray(wq_c.T), 512)
        m["wkt"] = to_tiles(np.ascontiguousarray(wk_c.T), HD)
        m["wvt"] = to_tiles(np.ascontiguousarray(wv_c.T), HD)
        m["wot"] = to_tiles(np.ascontiguousarray(wo_c.T), 512)
        in_maps.append(m)
    return in_maps


def kernel(x, cos, sin, wq, wk, wv, wo, _trace=False):
    x = np.asarray(x, np.float32)
    cos = np.asarray(cos, np.float32)
    sin = np.asarray(sin, np.float32)
    wq = np.asarray(wq, np.float32)
    wk = np.asarray(wk, np.float32)
    wv = np.asarray(wv, np.float32)
    wo = np.asarray(wo, np.float32)

    in_maps = _host_prep(x, cos, sin, wq, wk, wv, wo)
    if "nc" not in _NC_CACHE:
        _NC_CACHE["nc"] = _build()
    nc = _NC_CACHE["nc"]
    res = run_bass_kernel_spmd(
        nc, in_maps, core_ids=list(range(N_CORES)), trace=_trace
    )
    out = np.concatenate([res.results[c]["out"] for c in range(N_CORES)], axis=1)
    out = np.ascontiguousarray(out, dtype=np.float32)
    if _trace:
        kernel._last_exec_time_ns = res.exec_time_ns
        kernel._last_result = res
    return out


# revision 37
# speedup vs baseline: 1.1370x; 1.0247x over previous
"""GQA attention (S=2048, D=4096, 32 Q heads / 8 KV heads, RoPE, full attn)
distributed over 8 Trainium2 NeuronCores.

Strategy (tensor-parallel by heads; AllGather of normalized attention before
the output projection):
  - core c owns Q heads 4c..4c+3 and KV head c (GQA groups align with cores).
  - all GEMMs bf16 (stationary and moving), f32 PSUM accumulation.
  - projections as transposed GEMMs QT/KT/VT [chan, tok]: staggered 32-MM
    single-bank sweeps (one PSUM bank per output; banks release one at a
    time so RoPE drains overlap the next sweep); chunks 0-2 use a 4-bank
    ring, chunk 3's K/V + Q sweeps share a 2-bank ring with the
    attention-chunk-0 fillers.
  - RoPE via DVE muls (u=p*cs1, v=p*cs2) + deferred PE mix-matmuls flushed
    mid-next-sweep (never blocks the PE on the DVE); V transposed by
    SBUF->SBUF transposing DMAs.
  - attention per 512-token q-chunk, per head: scores ST=[k,q] per k-tile
    (3-bank ring), exp on ScalarE -> bf16, z on DVE (running f32 sum +
    ones-matmul partition reduce), PV accumulated over 16 k-tiles;
    normalize (1/z via ScalarE ln/exp, ones-broadcast matmul, DVE mul)
    deferred into the NEXT head's kt stream so the PE never waits.
  - the attention inner loop is exp-paced, so independent filler matmuls
    (wo-GEMM quarters of earlier chunks / last chunk's Q sweeps) fill the
    in-order PE queue between score/PV pairs.
  - AllGather per (chunk, head-pair) for chunks 0-2 ([256,512]bf16 ->
    [2048,512]) and per HEAD for chunk 3 ([128,512] -> [1024,512]) so the
    tail wo GEMM never waits; gathered tiles are DMA'd in half-waves
    pre-issued as soon as each collective is launched.
Host side only reshapes/transposes/casts inputs and concatenates outputs.
"""
import sys

import numpy as np
import ml_dtypes

_BF16 = ml_dtypes.bfloat16

for _p in ("/root/.axon_site/_ro/trn_rl_repo", "/opt/trn_rl_repo"):
    if _p not in sys.path:
        sys.path.append(_p)

import concourse.bass as bass
import concourse.tile as tile
from concourse import mybir
from concourse.bass_utils import run_bass_kernel_spmd

N_CORES = 8
S = 2048
D = 4096
HD = 128
N_QH = 4          # Q heads per core
N_KT = S // 128   # 16 k-tiles
N_TC = S // 512   # 4 token chunks
N_KC = D // 128   # 32 contraction tiles
F32 = mybir.dt.float32
BF = mybir.dt.bfloat16

_NC_CACHE = {}


def _split_multi_waits(nc):
    """This container's walrus accepts only ONE sync-wait per instruction
    encoding; hoist extra waits onto fresh single-wait NoOps placed before
    the instruction on the same engine."""
    n = 0
    for fn in nc.m.functions:
        for bb in fn.blocks:
            new_insts = []
            changed = False
            for ins in bb.instructions:
                si = ins.sync_info
                waits = list(si.on_wait) if si is not None else []
                if len(waits) > 1:
                    for w in waits[:-1]:
                        n += 1
                        nop = mybir.InstNoOp(name=f"WSPL-{n}", ins=[], outs=[])
                        nop.engine = ins.engine
                        nop.sync_info = mybir.SyncInfo(on_wait=[w], on_update=[])
                        new_insts.append(nop)
                    si.on_wait = waits[-1:]
                    changed = True
                new_insts.append(ins)
            if changed:
                bb.instructions = new_insts
    return n


def _build():
    nc = bass.Bass()

    xt = nc.dram_tensor("xt", [N_KC, 128, S], BF, kind="ExternalInput")
    wqt = nc.dram_tensor("wqt", [128, N_KC, 512], BF, kind="ExternalInput")
    wkt = nc.dram_tensor("wkt", [128, N_KC, HD], BF, kind="ExternalInput")
    wvt = nc.dram_tensor("wvt", [128, N_KC, HD], BF, kind="ExternalInput")
    wot = nc.dram_tensor("wot", [128, N_KC, 512], BF, kind="ExternalInput")
    cs1 = nc.dram_tensor("cs1", [HD, S], BF, kind="ExternalInput")
    cs2 = nc.dram_tensor("cs2", [HD, S], BF, kind="ExternalInput")
    mix1 = nc.dram_tensor("mix1", [HD, HD], BF, kind="ExternalInput")
    mix2 = nc.dram_tensor("mix2", [HD, HD], BF, kind="ExternalInput")
    onesc = nc.dram_tensor("onesc", [HD, 1], F32, kind="ExternalInput")
    onesr = nc.dram_tensor("onesr", [1, HD], BF, kind="ExternalInput")
    out_ext = nc.dram_tensor("out", [S, 512], F32, kind="ExternalOutput")

    # chunks 1-2: AllGather per head-pair; chunks 0 and 3: per head
    # (chunk 0's gathers must launch early to hide the ~50us wall latency
    #  during pipeline fill; chunk 3's must finish early for the tail)
    ag_in = {}
    ag_out = {}
    for qc in range(1, 3):
        for hp in range(2):
            ag_in[(qc, hp)] = nc.dram_tensor(f"agi{qc}_{hp}", [256, 512], BF)
            ag_out[(qc, hp)] = nc.dram_tensor(
                f"ago{qc}_{hp}", [2048, 512], BF, addr_space="Shared"
            )
    for qc in (0, 3):
        for h in range(N_QH):
            ag_in[(qc, h)] = nc.dram_tensor(f"agi{qc}h{h}", [128, 512], BF)
            ag_out[(qc, h)] = nc.dram_tensor(
                f"ago{qc}h{h}", [1024, 512], BF, addr_space="Shared"
            )

    with tile.TileContext(nc) as tc:
        with (
            tc.tile_pool(name="const", bufs=1) as constp,
            tc.tile_pool(name="persist", bufs=1) as persist,
            tc.tile_pool(name="xtp", bufs=10) as xtp,
            tc.tile_pool(name="uv", bufs=2) as uvp,
            tc.tile_pool(name="vt", bufs=2) as vtp,
            tc.tile_pool(name="ep", bufs=4) as ep,
            tc.tile_pool(name="zp", bufs=2) as zpool,
            tc.tile_pool(name="small", bufs=2) as smallp,
            tc.tile_pool(name="at", bufs=3) as atp,
            tc.tile_pool(name="rhs", bufs=7) as rhsp,
            tc.tile_pool(name="fout", bufs=2) as foutp,
        ):
            # ---- constants ----
            cs1_sb = constp.tile([HD, S], BF)
            cs2_sb = constp.tile([HD, S], BF)
            mix1_sb = constp.tile([HD, HD], BF)
            mix2_sb = constp.tile([HD, HD], BF)
            onesc_sb = constp.tile([HD, 1], F32)
            onesr_sb = constp.tile([1, HD], BF)
            nc.gpsimd.dma_start(out=cs1_sb[:], in_=cs1[:])
            nc.gpsimd.dma_start(out=cs2_sb[:], in_=cs2[:])
            nc.gpsimd.dma_start(out=mix1_sb[:], in_=mix1[:])
            nc.gpsimd.dma_start(out=mix2_sb[:], in_=mix2[:])
            nc.gpsimd.dma_start(out=onesc_sb[:], in_=onesc[:])
            nc.gpsimd.dma_start(out=onesr_sb[:], in_=onesr[:])

            # ---- persistent activations ----
            qt_sb = persist.tile([128, N_QH, S], BF)
            kt_sb = persist.tile([128, S], BF)
            v_sb = persist.tile([128, N_KT, HD], BF)

            xt_tiles = {}   # (c, g) -> sbuf tile [128, 4, 512]

            def load_xt_group(c, g):
                t = xtp.tile([128, 4, 512], BF, name=f"xt{c}_{g}", tag="xt")
                nc.sync.dma_start(
                    out=t[:],
                    in_=xt[g * 4:(g + 1) * 4, :, c * 512:(c + 1) * 512].rearrange(
                        "g p n -> p g n"
                    ),
                )
                xt_tiles[(c, g)] = t

            for g in range(8):
                load_xt_group(0, g)

            # rope stage 2 (PE mix matmuls + copy) is deferred and flushed
            # mid-next-sweep, so the PE never waits on the DVE muls.
            pending_rope = []

            def flush_rope(pool):
                while pending_rope:
                    u, v, dst, key = pending_rope.pop(0)
                    rps = pool.tile([128, 512], F32, name=f"rps{key}", tag=pool._ropetag)
                    nc.tensor.matmul(rps[:], mix1_sb[:], u[:], start=True, stop=False)
                    nc.tensor.matmul(rps[:], mix2_sb[:], v[:], start=False, stop=True)
                    nc.scalar.copy(dst, rps[:])

            def rope_stage1(acc, dst, t0, key):
                u = uvp.tile([128, 512], BF, name=f"u{key}", tag="u")
                v = uvp.tile([128, 512], BF, name=f"v{key}", tag="v")
                nc.vector.tensor_mul(u[:], acc[:], cs1_sb[:, t0:t0 + 512])
                nc.vector.tensor_mul(v[:], acc[:], cs2_sb[:, t0:t0 + 512])
                pending_rope.append((u, v, dst, key))

            def proj_drain(acc, c, kind, h):
                t0 = c * 512
                if kind == "q":
                    rope_stage1(acc, qt_sb[:, h, t0:t0 + 512], t0, f"q{c}_{h}")
                elif kind == "k":
                    rope_stage1(acc, kt_sb[:, t0:t0 + 512], t0, f"k{c}")
                else:
                    vt_t = vtp.tile([128, 512], BF, name=f"vt{c}", tag="vt")
                    nc.scalar.copy(vt_t[:], acc[:])
                    for g in range(4):
                        nc.sync.dma_start_transpose(
                            out=v_sb[:, c * 4 + g, :],
                            in_=vt_t[:, g * 128:(g + 1) * 128],
                        )

            def wslice_fn(kind, h):
                if kind == "q":
                    return lambda kc: wq_sb[:, kc, h * 128:(h + 1) * 128]
                if kind == "k":
                    return lambda kc: wk_sb[:, kc, :]
                return lambda kc: wv_sb[:, kc, :]

            def proj_sweep(pp, rope_pool, c, kind, h=0):
                """One 32-MM projection sweep into one PSUM bank."""
                acc = pp.tile([128, 512], F32, name=f"acc_{kind}{c}_{h}", tag="pacc")
                ws = wslice_fn(kind, h)
                for kc in range(N_KC):
                    if kc == 8:
                        flush_rope(rope_pool)
                    nc.tensor.matmul(
                        acc[:], ws(kc), xt_tiles[(c, kc // 4)][:, kc % 4, :],
                        start=(kc == 0), stop=(kc == N_KC - 1),
                    )
                proj_drain(acc, c, kind, h)

            def proj_sweep_gen(pp, rope_pool, c, kind, h=0, tag="pacc"):
                """proj_sweep as a filler generator: 16 yields of 2 MMs."""
                acc = pp.tile([128, 512], F32, name=f"acc_{kind}{c}_{h}", tag=tag)
                ws = wslice_fn(kind, h)
                for kp in range(16):
                    if kp == 4:
                        flush_rope(rope_pool)
                    for kc in (2 * kp, 2 * kp + 1):
                        nc.tensor.matmul(
                            acc[:], ws(kc), xt_tiles[(c, kc // 4)][:, kc % 4, :],
                            start=(kc == 0), stop=(kc == N_KC - 1),
                        )
                    if kp < 15:
                        yield
                proj_drain(acc, c, kind, h)
                yield

            # ---------- wo GEMM machinery ----------
            wo_state = {}
            cur_pools = {}   # phase-scoped PSUM pools for attention

            def load_rhs_halves(qc, hp):
                """DMA one gathered wave into two [128,8,512] half tiles."""
                halves = []
                for half in range(2):
                    r = rhsp.tile(
                        [128, 8, 512], BF, name=f"rhs{qc}_{hp}_{half}", tag="rhs"
                    )
                    nc.sync.dma_start(
                        out=r[:],
                        in_=ag_out[(qc, hp)][half * 1024:(half + 1) * 1024, :]
                        .rearrange("(t p) n -> p t n", p=128),
                    )
                    halves.append(r)
                wo_state[(qc, hp)] = halves

            def load_rhs_head(qc, h):
                r = rhsp.tile([128, 8, 512], BF, name=f"rhs{qc}h{h}", tag="rhs")
                nc.sync.dma_start(
                    out=r[:],
                    in_=ag_out[(qc, h)].rearrange("(t p) n -> p t n", p=128),
                )
                wo_state[(qc, h)] = r

            def wo_quarter_gen(qc, wop, quarter):
                """wo GEMM for chunk qc (0..2), one quarter: 16 yields x 2 MMs.
                quarter 0: fps01 += wave0      quarter 1: fps01 += wave1, fout
                quarter 2: fps23 += wave0      quarter 3: fps23 += wave1, fout"""
                q0 = qc * 512
                hp = quarter % 2
                qsp = quarter // 2
                if qc == 0:
                    halves = [wo_state[(0, 2 * hp)], wo_state[(0, 2 * hp + 1)]]
                else:
                    halves = wo_state[(qc, hp)]
                if hp == 0:
                    fps = [
                        wop.tile(
                            [128, 512], F32, name=f"f{qc}_{qsp}_{j}", tag=f"f{j}"
                        )
                        for j in range(2)
                    ]
                    wo_state[(qc, "fps", qsp)] = fps
                fps = wo_state[(qc, "fps", qsp)]
                for i in range(16):
                    ci, jj = i // 2, i % 2
                    hk = 4 * ci + 2 * hp + jj
                    if qc == 0:
                        rhs = halves[jj]
                        li = ci
                    else:
                        rhs = halves[0] if ci < 4 else halves[1]
                        li = (ci % 4) * 2 + jj
                    for j in range(2):
                        qs = qsp * 2 + j
                        nc.tensor.matmul(
                            fps[j][:],
                            rhs[:, li, qs * 128:(qs + 1) * 128],
                            wo_sb[:, hk, :],
                            start=(hp == 0 and i == 0),
                            stop=(hp == 1 and i == 15),
                        )
                    if i < 15:
                        yield
                if hp == 1:
                    for j in range(2):
                        qs = qsp * 2 + j
                        f_sb = foutp.tile(
                            [128, 512], F32, name=f"fs{qc}_{qsp}_{j}", tag="fs"
                        )
                        nc.scalar.copy(f_sb[:], fps[j][:])
                        nc.sync.dma_start(
                            out=out_ext[q0 + qs * 128:q0 + (qs + 1) * 128, :],
                            in_=f_sb[:],
                        )
                yield

            def wo3_pass(wop, qsp):
                """Last chunk: one fps pair accumulated across 4 head-waves."""
                q0 = 3 * 512
                fps = [
                    wop.tile([128, 512], F32, name=f"f3_{qsp}_{j}", tag=f"f{j}")
                    for j in range(2)
                ]
                for h in range(N_QH):
                    rhs = wo_state[(3, h)]
                    for ci in range(8):
                        hk = 4 * ci + h
                        for j in range(2):
                            qs = qsp * 2 + j
                            nc.tensor.matmul(
                                fps[j][:],
                                rhs[:, ci, qs * 128:(qs + 1) * 128],
                                wo_sb[:, hk, :],
                                start=(h == 0 and ci == 0),
                                stop=(h == N_QH - 1 and ci == 7),
                            )
                for j in range(2):
                    qs = qsp * 2 + j
                    f_sb = foutp.tile(
                        [128, 512], F32, name=f"fs3_{qsp}_{j}", tag="fs"
                    )
                    nc.scalar.copy(f_sb[:], fps[j][:])
                    nc.sync.dma_start(
                        out=out_ext[q0 + qs * 128:q0 + (qs + 1) * 128, :],
                        in_=f_sb[:],
                    )

            def run_filler(f, n=1):
                if f is None:
                    return
                for _ in range(n):
                    try:
                        next(f)
                    except StopIteration:
                        break

            def ag_launch(qc, part):
                nc.gpsimd.collective_compute(
                    "AllGather",
                    mybir.AluOpType.bypass,
                    replica_groups=[list(range(N_CORES))],
                    ins=[ag_in[(qc, part)][:].opt()],
                    outs=[ag_out[(qc, part)][:].opt()],
                )

            class NormTail:
                """Deferred per-head softmax normalization, emitted inside the
                NEXT head's kt stream (kt2: z-reduce + 1/z; kt8: broadcast,
                normalize, store, collective launch + gathered-wave DMA)."""

                def __init__(self, qc, h, pv, z_acc):
                    self.qc, self.h, self.pv, self.z_acc = qc, h, pv, z_acc

                def stage_a(self):
                    qc, h = self.qc, self.h
                    zr = cur_pools["pzb"].tile([1, 512], F32, name=f"zr{qc}_{h}", tag="zb")
                    nc.tensor.matmul(
                        zr[:], onesc_sb[:], self.z_acc[:], start=True, stop=True
                    )
                    lnz = smallp.tile([1, 512], F32, name=f"ln{qc}_{h}", tag="lnz")
                    nc.scalar.activation(
                        out=lnz[:], in_=zr[:],
                        func=mybir.ActivationFunctionType.Ln,
                    )
                    self.invz = smallp.tile([1, 512], BF, name=f"iz{qc}_{h}", tag="iz")
                    nc.scalar.activation(
                        out=self.invz[:], in_=lnz[:],
                        func=mybir.ActivationFunctionType.Exp, scale=-1.0,
                    )

                def stage_b(self):
                    qc, h = self.qc, self.h
                    bc = cur_pools["pzb"].tile([128, 512], F32, name=f"bc{qc}_{h}", tag="zb")
                    nc.tensor.matmul(
                        bc[:], onesr_sb[:], self.invz[:], start=True, stop=True
                    )
                    bc_sb = smallp.tile([128, 512], BF, name=f"bcs{qc}_{h}", tag="bcs")
                    nc.scalar.copy(bc_sb[:], bc[:])
                    at_sb = atp.tile([128, 512], BF, name=f"at{qc}_{h}", tag="at")
                    nc.vector.tensor_mul(at_sb[:], self.pv[:], bc_sb[:])
                    if qc in (1, 2):
                        hp, j = h // 2, h % 2
                        nc.gpsimd.dma_start(
                            out=ag_in[(qc, hp)][j * 128:(j + 1) * 128, :],
                            in_=at_sb[:],
                        )
                        if j == 1:
                            ag_launch(qc, hp)
                            if hp == 1:
                                # wave1 halves: ring slot is free by now
                                load_rhs_halves(qc, 1)
                    else:
                        nc.gpsimd.dma_start(out=ag_in[(qc, h)][:], in_=at_sb[:])
                        ag_launch(qc, h)
                        load_rhs_head(qc, h)

            def emit_ev(qc, h, kt, stps, pv, z_acc):
                e_t = ep.tile([128, 512], BF, name=f"e{qc}_{h}_{kt}", tag="e")
                nc.scalar.activation(
                    out=e_t[:], in_=stps[kt][:],
                    func=mybir.ActivationFunctionType.Exp,
                )
                nc.tensor.matmul(
                    pv[:], v_sb[:, kt, :], e_t[:],
                    start=(kt == 0), stop=(kt == N_KT - 1),
                )
                if kt == 0:
                    nc.vector.tensor_copy(z_acc[:], e_t[:])
                else:
                    nc.vector.tensor_add(z_acc[:], z_acc[:], e_t[:])

            def attn_head(qc, h, filler=None, prev_tail=None):
                q0 = qc * 512
                pv = cur_pools["ppv"].tile([128, 512], F32, name=f"pv{qc}_{h}", tag=f"pv{h % 2}")
                z_acc = zpool.tile([128, 512], F32, name=f"z{qc}_{h}", tag="zacc")
                stps = {}
                for kt in range(N_KT):
                    if prev_tail is not None:
                        if kt == 2:
                            prev_tail.stage_a()
                        elif kt == 8:
                            prev_tail.stage_b()
                    stp = cur_pools["psc"].tile([128, 512], F32, name=f"st{qc}_{h}_{kt}", tag="st")
                    stps[kt] = stp
                    nc.tensor.matmul(
                        stp[:],
                        kt_sb[:, kt * 128:(kt + 1) * 128],
                        qt_sb[:, h, q0:q0 + 512],
                        start=True, stop=True,
                    )
                    run_filler(filler)
                    if kt > 0:
                        emit_ev(qc, h, kt - 1, stps, pv, z_acc)
                emit_ev(qc, h, N_KT - 1, stps, pv, z_acc)
                run_filler(filler)
                return NormTail(qc, h, pv, z_acc)

            # ================= phase 1: projections chunks 0-2 =============
            with tc.tile_pool(name="wqp", bufs=1) as wqp:
                wk_sb = wqp.tile([128, N_KC, HD], BF)
                wv_sb = wqp.tile([128, N_KC, HD], BF)
                nc.sync.dma_start(out=wk_sb[:], in_=wkt[:])
                nc.sync.dma_start(out=wv_sb[:], in_=wvt[:])
                wq_sb = wqp.tile([128, N_KC, 512], BF)
                for ch in range(4):
                    nc.sync.dma_start(
                        out=wq_sb[:, ch * 8:(ch + 1) * 8, :],
                        in_=wqt[:, ch * 8:(ch + 1) * 8, :],
                    )
                with tc.tile_pool(name="pprojA", bufs=4, space="PSUM") as pprojA:
                    pprojA._ropetag = "pacc"
                    for c in range(3):
                        scope = nc.named_scope(f"proj{c}"); scope.__enter__()
                        load_xt_group(c + 1, 0)
                        proj_sweep(pprojA, pprojA, c, "k")
                        load_xt_group(c + 1, 1)
                        proj_sweep(pprojA, pprojA, c, "v")
                        load_xt_group(c + 1, 2)
                        load_xt_group(c + 1, 3)
                        for hh in range(N_QH):
                            proj_sweep(pprojA, pprojA, c, "q", hh)
                            load_xt_group(c + 1, 4 + hh)
                        scope.__exit__(None, None, None)

                # ============ phase 2: c3 K/V + attention chunk 0 ==========
                # (pprojA closed; fresh 8-bank layout: psc 3 + ppv 2 + pzb 1
                #  + pprojB 2)
                with (
                    tc.tile_pool(name="pscA", bufs=3, space="PSUM") as psc,
                    tc.tile_pool(name="ppvA", bufs=1, space="PSUM") as ppv,
                    tc.tile_pool(name="pzbA", bufs=1, space="PSUM") as pzb,
                    tc.tile_pool(name="pprojB", bufs=2, space="PSUM") as pprojB,
                ):
                    pzb._ropetag = "zb"
                    cur_pools["psc"], cur_pools["ppv"], cur_pools["pzb"] = (
                        psc, ppv, pzb
                    )
                    scope = nc.named_scope("proj3"); scope.__enter__()
                    proj_sweep(pprojB, pzb, 3, "k")
                    proj_sweep(pprojB, pzb, 3, "v")
                    scope.__exit__(None, None, None)

                    scope = nc.named_scope("attn0"); scope.__enter__()
                    tail = None
                    for h in range(N_QH):
                        g = proj_sweep_gen(pprojB, pzb, 3, "q", h)
                        tail = attn_head(0, h, filler=g, prev_tail=tail)
                    # flush the last head's tail inside this pool scope
                    tail.stage_a()
                    tail.stage_b()
                    tail = None
                    flush_rope(pzb)
                    scope.__exit__(None, None, None)

            # ======== phase 3: attention chunks 1-3 + wo ===========
            with (
                tc.tile_pool(name="wop", bufs=1) as wop_s,
                tc.tile_pool(name="pscB", bufs=3, space="PSUM") as psc,
                tc.tile_pool(name="ppvB", bufs=1, space="PSUM") as ppv,
                tc.tile_pool(name="pzbB", bufs=1, space="PSUM") as pzb,
                tc.tile_pool(name="pwo", bufs=1, space="PSUM") as pwo,
            ):
                cur_pools["psc"], cur_pools["ppv"], cur_pools["pzb"] = (
                    psc, ppv, pzb
                )
                pzb._ropetag = "zb"
                wo_sb = wop_s.tile([128, N_KC, 512], BF)
                for ch in range(4):
                    nc.sync.dma_start(
                        out=wo_sb[:, ch * 8:(ch + 1) * 8, :],
                        in_=wot[:, ch * 8:(ch + 1) * 8, :],
                    )
                pending = [None, None]
                for qc in range(1, N_TC):
                    scope = nc.named_scope(f"attn{qc}"); scope.__enter__()
                    if qc >= 2:
                        pending.append(wo_quarter_gen(qc - 2, pwo, 3))
                    for q in range(3):
                        pending.append(wo_quarter_gen(qc - 1, pwo, q))
                    for h in range(N_QH):
                        f = pending.pop(0) if pending else None
                        tail = attn_head(qc, h, filler=f, prev_tail=tail)
                        run_filler(f, 16)
                    if qc < 3:
                        load_rhs_halves(qc, 0)
                    scope.__exit__(None, None, None)
                scope = nc.named_scope("wo3"); scope.__enter__()
                tail.stage_a()
                tail.stage_b()
                pending.append(wo_quarter_gen(N_TC - 2, pwo, 3))
                for f in pending:
                    run_filler(f, 17)
                wo3_pass(pwo, 0)
                wo3_pass(pwo, 1)
                scope.__exit__(None, None, None)

    _split_multi_waits(nc)
    return nc


def _host_prep(x, cos, sin, wq, wk, wv, wo):
    scale = np.float32(HD ** -0.5)
    perm = np.concatenate([np.arange(0, HD, 2), np.arange(1, HD, 2)])

    xt = np.ascontiguousarray(x.T.reshape(N_KC, 128, S)).astype(_BF16)
    cosT = cos.T.astype(np.float32)
    sinT = sin.T.astype(np.float32)
    cs1 = np.concatenate([cosT, sinT], axis=0).astype(_BF16)
    cs2 = np.concatenate([sinT, cosT], axis=0).astype(_BF16)

    m1 = np.zeros((HD, HD), np.float32)
    m1[np.arange(64), np.arange(64)] = 1.0
    m1[np.arange(64) + 64, np.arange(64)] = -1.0
    m2 = np.zeros((HD, HD), np.float32)
    m2[np.arange(64), np.arange(64) + 64] = 1.0
    m2[np.arange(64) + 64, np.arange(64) + 64] = 1.0

    def to_tiles(wT, ncols):
        return np.ascontiguousarray(
            wT.reshape(N_KC, 128, ncols).transpose(1, 0, 2)
        ).astype(_BF16)

    shared = {
        "xt": xt,
        "cs1": cs1,
        "cs2": cs2,
        "mix1": m1.astype(_BF16),
        "mix2": m2.astype(_BF16),
        "onesc": np.ones((HD, 1), np.float32),
        "onesr": np.ones((1, HD), np.float32).astype(_BF16),
    }
    in_maps = []
    for c in range(N_CORES):
        wq_c = wq[c * 512:(c + 1) * 512].reshape(N_QH, HD, D)[:, perm, :]
        wq_c = wq_c.reshape(512, D) * scale
        wk_c = wk[c * HD:(c + 1) * HD][perm, :]
        wv_c = wv[c * HD:(c + 1) * HD]
        wo_c = wo[c * 512:(c + 1) * 512]
        m = dict(shared)
        m["wqt"] = to_tiles(np.ascontiguousarray(wq_c.T), 512)
        m["wkt"] = to_tiles(np.ascontiguousarray(wk_c.T), HD)
        m["wvt"] = to_tiles(np.ascontiguousarray(wv_c.T), HD)
        m["wot"] = to_tiles(np.ascontiguousarray(wo_c.T), 512)
        in_maps.append(m)
    return in_maps


def kernel(x, cos, sin, wq, wk, wv, wo, _trace=False):
    x = np.asarray(x, np.float32)
    cos = np.asarray(cos, np.float32)
    sin = np.asarray(sin, np.float32)
    wq = np.asarray(wq, np.float32)
    wk = np.asarray(wk, np.float32)
    wv = np.asarray(wv, np.float32)
    wo = np.asarray(wo, np.float32)

    in_maps = _host_prep(x, cos, sin, wq, wk, wv, wo)
    if "nc" not in _NC_CACHE:
        _NC_CACHE["nc"] = _build()
    nc = _NC_CACHE["nc"]
    res = run_bass_kernel_spmd(
        nc, in_maps, core_ids=list(range(N_CORES)), trace=_trace
    )
    out = np.concatenate([res.results[c]["out"] for c in range(N_CORES)], axis=1)
    out = np.ascontiguousarray(out, dtype=np.float32)
    if _trace:
        kernel._last_exec_time_ns = res.exec_time_ns
        kernel._last_result = res
    return out
